# revision 13
# baseline (speedup 1.0000x reference)
"""BiLSTM-CRF Trainium2 kernel: 8-core SPMD, chunk-parallel LSTM + Viterbi.

Strategy (validated numerically against the reference in float32):
- Core k owns output slice [k*256, (k+1)*256) of the T=2048 sequence.
- The LSTM recurrence is chunk-parallelized: per core, per direction, 32
  streams each process a 14-step chunk preceded by a 40-step warmup from zero
  state (state influence decays ~4x/step, so the warmup converges to f32
  noise). Streams are batched in the matmul free dim, so one "superstep" does
  16 bf16 weight-tile matmuls for all 32 streams at once.
- Viterbi alpha (forward) and beta (backward) max-plus scans are
  chunk-parallelized the same way (128 streams x 3 steps, 56-step coalescence
  warmup); path[t] = argmax_i(alpha_t[i] + beta_t[i]).
- The score is recomputed from the decoded path (sum of edge scores) because
  chunked scans lose the global additive constant; per-core partials are
  summed on the host (part of unsharding).
- Sequence edges are handled with data only (all 8 cores run one program):
  virtual rows outside [0,T) use a zero word-embedding row and a pad char id;
  the alpha START init and the beta STOP init are injected via patched feats
  rows and a masked replicated transition matrix on the affected cores.

kernel(**inputs) -> (score, path) matching reference.reference().
"""
import numpy as np
from contextlib import ExitStack

import concourse.bass as bass
import concourse.tile as tile
from concourse import bacc, mybir

F32 = mybir.dt.float32
F32R = mybir.dt.float32r
BF16 = mybir.dt.bfloat16
I32 = mybir.dt.int32
AF = mybir.ActivationFunctionType
ALU = mybir.AluOpType
AX = mybir.AxisListType

# ---------------- geometry ----------------
T, K, H, E, CO, CE, CV, L = 2048, 12, 256, 300, 25, 25, 85, 20
V = 100000
START, STOP = 10, 11
NEG = -10000.0
NC = 8
B = T // NC          # 256

W = 40               # LSTM warmup steps
WV = 56              # Viterbi warmup steps
S = 32               # LSTM streams per direction
BF = 448             # feats rows per core; feats row r <-> t = k*B - 57 + r
B2 = BF // S         # 14
NSUP = B2 + W        # 54 LSTM supersteps
PR = BF + 2 * W      # 528 precomp rows; precomp row pr = feats row + W
TEXT = 640           # padded emb/precomp columns (5*128)
LP = 24              # padded word length
NCH = TEXT * LP
B2V = 3
NSUPV = WV + B2V     # 59
FPAD = 80            # junk rows below feats row 0 in DRAM buffers
FD = 608             # DRAM rows in feats/ab buffers (FPAD + 512 + 16)
PADCHAR = 200.0


def row2t(k, r):     # feats row -> global t
    return k * B - (WV + 1) + r


def pr2t(k, pr):     # precomp/emb column -> global t
    return k * B - (WV + 1 + W) + pr


def _dap(dram, offset, pairs):
    """Raw strided AP over a DRAM tensor (element units)."""
    return bass.AP(dram, offset, [list(p) for p in pairs])


def build_nc(debug_outputs=False):
    nc = bacc.Bacc("TRN2", target_bir_lowering=False, debug=False, num_devices=NC)
    d = {}

    def ein(n, sh, dt):
        d[n] = nc.dram_tensor(n, sh, dt, kind="ExternalInput")

    ein("word_emb", [V + 1, E], F32)
    ein("widx", [TEXT, 1], I32)
    ein("charsb", [NCH], BF16)
    ein("iota85", [CV, 1], F32)
    ein("ident", [128, 128], F32)
    ein("w_ihT", [128, 2 * 3 * 8 * 128], F32)
    ein("w_hhT", [128, 2 * 2 * 8 * 128], BF16)
    ein("h2tT", [128, 2 * 2 * 16], F32)
    ein("cembT", [CE, CV], F32)
    ein("convT", [CE, 3 * CO], F32)
    ein("convb", [CO, 1], F32)
    ein("trrep", [128, K * K], F32)
    ein("trrepb", [128, K * K], F32)
    ein("fmask", [FD, K], F32)
    ein("fpatch", [FD, K], F32)
    ein("trstop", [128, K], F32)
    ein("vmask1", [128, 3], F32)
    ein("vmask2", [128, 3], F32)
    ein("vmask3", [128, 1], F32)

    path_o = nc.dram_tensor("path_o", [B], I32, kind="ExternalOutput")
    score_o = nc.dram_tensor("score_o", [128, 1], F32, kind="ExternalOutput")
    dbg = {}
    if debug_outputs:
        dbg["feats_o"] = nc.dram_tensor("feats_o", [FD, K], F32, kind="ExternalOutput")
        dbg["emb_o"] = nc.dram_tensor("emb_o", [3, 128, TEXT], F32, kind="ExternalOutput")
        dbg["h_o"] = nc.dram_tensor("h_o", [2, 128, 2 * BF], F32, kind="ExternalOutput")
        dbg["ab_o"] = nc.dram_tensor("ab_o", [2, FD, K], F32, kind="ExternalOutput")

    feats_d = nc.dram_tensor("feats_d", [FD, K], F32, kind="Internal")
    ab_d = nc.dram_tensor("ab_d", [2, FD, K], F32, kind="Internal")

    with tile.TileContext(nc) as tc, ExitStack() as ctx:
        sb = ctx.enter_context(tc.tile_pool(name="sb", bufs=1))

        # ---------- P0: params ----------
        w_ihT = sb.tile([128, 2 * 3 * 8 * 128], F32R)
        nc.gpsimd.dma_start(w_ihT[:], d["w_ihT"].ap())
        w_hhT = sb.tile([128, 2 * 2 * 8 * 128], BF16)
        nc.sync.dma_start(w_hhT[:], d["w_hhT"].ap())
        h2tT = sb.tile([128, 2 * 2 * 16], F32)
        nc.sync.dma_start(h2tT[:], d["h2tT"].ap())
        ident = sb.tile([128, 128], F32)
        nc.sync.dma_start(ident[:], d["ident"].ap())
        trrep = sb.tile([128, K * K], F32)
        nc.sync.dma_start(trrep[:], d["trrep"].ap())
        trrepb = sb.tile([128, K * K], F32)
        nc.sync.dma_start(trrepb[:], d["trrepb"].ap())
        trstop = sb.tile([128, K], F32)
        nc.sync.dma_start(trstop[:], d["trstop"].ap())
        vmask1 = sb.tile([128, 3], F32)
        nc.sync.dma_start(vmask1[:], d["vmask1"].ap())
        vmask2 = sb.tile([128, 3], F32)
        nc.sync.dma_start(vmask2[:], d["vmask2"].ap())
        vmask3 = sb.tile([128, 1], F32)
        nc.sync.dma_start(vmask3[:], d["vmask3"].ap())

        embT = [sb.tile([128, TEXT], F32, tag=f"embT{c}", name=f"embT{c}") for c in range(3)]
        embTr = [sb.tile([128, TEXT], F32R, tag=f"embTr{c}", name=f"embTr{c}") for c in range(3)]
        precompT = [sb.tile([128, 8 * TEXT], F32, tag=f"pre{dd}", name=f"pre{dd}") for dd in range(2)]
        hval = [sb.tile([128, 2 * BF], F32, tag=f"hval{dd}", name=f"hval{dd}") for dd in range(2)]

        # ---------- P1-P4: embeddings + charCNN ----------
        with tc.tile_pool(name="psA", bufs=2, space="PSUM") as psA, \
             tc.tile_pool(name="sbA", bufs=1) as sbA:
            iota85 = sbA.tile([CV, 1], F32, tag="iota85")
            nc.sync.dma_start(iota85[:], d["iota85"].ap())
            cembT = sbA.tile([CE, CV], F32, tag="cembT")
            nc.sync.dma_start(cembT[:], d["cembT"].ap())
            convT = sbA.tile([CE, 3 * CO], F32, tag="convT")
            nc.sync.dma_start(convT[:], d["convT"].ap())
            convb = sbA.tile([128, 1], F32, tag="convb")
            nc.sync.dma_start(convb[64:64 + CO, :], d["convb"].ap())

            ctab_ps = psA.tile([CV, 3 * CO], F32, space="PSUM", tag="ctab")
            nc.tensor.matmul(ctab_ps[:], cembT[:], convT[:], start=True, stop=True)
            ctab = sbA.tile([CV, 3 * CO], BF16, tag="ctabsb")
            nc.vector.tensor_copy(ctab[:], ctab_ps[:])

            onehot = sbA.tile([CV, NCH], BF16, tag="onehot")
            CHCH = 3840
            for c0 in range(0, NCH, CHCH):
                chbc = sbA.tile([CV, CHCH], BF16, tag="chb", bufs=2, name=f"chb{c0}")
                nc.sync.dma_start(chbc[:], _dap(d["charsb"], c0, [[0, CV], [1, CHCH]]))
                nc.vector.tensor_scalar(out=onehot[:, c0:c0 + CHCH], in0=chbc[:],
                                        scalar1=iota85[:], scalar2=None,
                                        op0=ALU.is_equal)

            for c in range(3):
                nc.vector.memset(embT[c][:], 0.0)

            widx = sbA.tile([128, 5], I32, tag="widx")
            nc.sync.dma_start(widx[:], _dap(d["widx"], 0, [[1, 128], [128, 5]]))
            for g in range(5):
                wrows = sbA.tile([128, 304], F32, tag="wrows", bufs=2, name=f"wrows{g}")
                nc.gpsimd.indirect_dma_start(
                    out=wrows[:, :300], out_offset=None, in_=d["word_emb"].ap(),
                    in_offset=bass.IndirectOffsetOnAxis(ap=widx[:, g:g + 1], axis=0))
                for c in range(3):
                    dim0 = c * 128
                    ncols = min(128, 300 - dim0)
                    tp = psA.tile([128, 128], F32, space="PSUM", tag="tp")
                    nc.tensor.transpose(tp[:ncols, :], wrows[:, dim0:dim0 + ncols],
                                        ident[:])
                    nc.vector.tensor_copy(embT[c][:ncols, g * 128:(g + 1) * 128],
                                          tp[:ncols, :128])
            nc.vector.memset(embT[2][96:97, :], 1.0)

            # charCNN: co[d,(t,j)] = sum_k Tk-matmul over windowed onehot
            TCW = 23
            oh3 = onehot[:].rearrange("c (t l) -> c t l", l=LP)
            nsl = (TEXT + TCW - 1) // TCW
            for sl in range(nsl):
                t0 = sl * TCW
                tn = min(TCW, TEXT - t0)
                co_ps = psA.tile([128, TCW * (L + 2)], F32, space="PSUM", tag="co")
                co3 = co_ps[64:64 + CO, :tn * (L + 2)].rearrange("d (t j) -> d t j", j=L + 2)
                for k3 in range(3):
                    nc.tensor.matmul(co3, ctab[:, k3 * CO:(k3 + 1) * CO],
                                     oh3[:, t0:t0 + tn, k3:k3 + L + 2],
                                     start=(k3 == 0), stop=(k3 == 2))
                cf = sbA.tile([128, TCW], F32, tag="cf", bufs=2, name=f"cf{sl}")
                nc.vector.tensor_reduce(cf[64:64 + CO, :tn], co3, axis=AX.X, op=ALU.max)
                nc.vector.tensor_scalar(out=embT[2][64:64 + CO, t0:t0 + tn],
                                        in0=cf[64:64 + CO, :tn],
                                        scalar1=convb[64:64 + CO, :],
                                        scalar2=None, op0=ALU.add)

            for c in range(3):
                nc.vector.tensor_copy(embTr[c][:], embT[c][:])

        # ---------- P5: input matmuls ----------
        with tc.tile_pool(name="psB", bufs=2, space="PSUM") as psB:
            for dd in range(2):
                for mc in range(8):
                    pc_ps = psB.tile([128, TEXT], F32, space="PSUM", tag="pcps")
                    for n0, nn in ((0, 512), (512, 128)):
                        for kc in range(3):
                            w0 = ((dd * 3 + kc) * 8 + mc) * 128
                            nc.tensor.matmul(pc_ps[:, n0:n0 + nn],
                                             w_ihT[:, w0:w0 + 128],
                                             embTr[kc][:, n0:n0 + nn],
                                             start=(kc == 0), stop=(kc == 2))
                    nc.scalar.activation(precompT[dd][:, mc * TEXT:(mc + 1) * TEXT],
                                         pc_ps[:], AF.Copy)

        # ---------- P6: LSTM supersteps ----------
        def pr_ap(dd, u):
            t3 = precompT[dd][:].rearrange("p (c r) -> p c r", c=8)
            if dd == 0:
                return t3[:, :, u:u + (S - 1) * B2 + 1:B2]
            base = PR - 1 - u
            return t3[:, :, base - (S - 1) * B2:base + 1:B2]

        def hv_ap(dd, u):
            t3 = hval[dd][:].rearrange("p (c r) -> p c r", c=2)
            if dd == 0:
                r0 = u - W
                return t3[:, :, r0:r0 + (S - 1) * B2 + 1:B2]
            base = BF - 1 - (u - W)
            return t3[:, :, base - (S - 1) * B2:base + 1:B2]

        with tc.tile_pool(name="psC", bufs=2, space="PSUM") as psC, \
             tc.tile_pool(name="sbC", bufs=1) as sbC:
            hscr = [sbC.tile([128, 2 * S], F32, tag=f"hscr{dd}", name=f"hscr{dd}") for dd in range(2)]
            hbf = [sbC.tile([128, 2 * S], BF16, tag=f"hbf{dd}", name=f"hbf{dd}") for dd in range(2)]
            cstA = [sbC.tile([128, 2 * S], F32, tag=f"cstA{dd}", name=f"cstA{dd}") for dd in range(2)]
            cstB = [sbC.tile([128, 2 * S], F32, tag=f"cstB{dd}", name=f"cstB{dd}") for dd in range(2)]
            gsb = [sbC.tile([128, 8 * S], F32, tag=f"gsb{dd}", name=f"gsb{dd}") for dd in range(2)]
            sgo = [sbC.tile([128, 6 * S], F32, tag=f"sgo{dd}", name=f"sgo{dd}") for dd in range(2)]
            tgg = [sbC.tile([128, 2 * S], F32, tag=f"tgg{dd}", name=f"tgg{dd}") for dd in range(2)]
            tg2 = [sbC.tile([128, 2 * S], F32, tag=f"tg2{dd}", name=f"tg2{dd}") for dd in range(2)]
            tm1 = [sbC.tile([128, 2 * S], F32, tag=f"tm1{dd}", name=f"tm1{dd}") for dd in range(2)]
            for dd in range(2):
                nc.vector.memset(hbf[dd][:], 0.0)
                nc.vector.memset(cstA[dd][:], 0.0)
                nc.vector.memset(hval[dd][:], 0.0)

            for u in range(NSUP):
                for dd in range(2):
                    gates = psC.tile([128, 8 * S], F32, space="PSUM", tag=f"g{dd}")
                    for mc in range(8):
                        for kc in range(2):
                            w0 = ((dd * 2 + kc) * 8 + mc) * 128
                            nc.tensor.matmul(gates[:, mc * S:(mc + 1) * S],
                                             w_hhT[:, w0:w0 + 128],
                                             hbf[dd][:, kc * S:(kc + 1) * S],
                                             start=(kc == 0), stop=(kc == 1))
                    g = gsb[dd]
                    nc.vector.tensor_tensor(
                        out=g[:].rearrange("p (c r) -> p c r", c=8),
                        in0=gates[:].rearrange("p (c r) -> p c r", c=8),
                        in1=pr_ap(dd, u), op=ALU.add)
                    nc.scalar.activation(sgo[dd][:, :4 * S], g[:, :4 * S], AF.Sigmoid)
                    nc.scalar.activation(sgo[dd][:, 4 * S:6 * S], g[:, 6 * S:8 * S],
                                         AF.Sigmoid)
                    nc.scalar.activation(tgg[dd][:], g[:, 4 * S:6 * S], AF.Sigmoid,
                                         scale=2.0)
                    nc.vector.tensor_scalar(out=tg2[dd][:], in0=tgg[dd][:],
                                            scalar1=2.0, scalar2=-1.0,
                                            op0=ALU.mult, op1=ALU.add)
                    nc.vector.tensor_mul(tm1[dd][:], sgo[dd][:, :2 * S], tg2[dd][:])
                    nc.vector.tensor_mul(cstB[dd][:], sgo[dd][:, 2 * S:4 * S],
                                         cstA[dd][:])
                    nc.vector.tensor_add(cstA[dd][:], cstB[dd][:], tm1[dd][:])
                    nc.scalar.activation(tgg[dd][:], cstA[dd][:], AF.Sigmoid, scale=2.0)
                    nc.vector.tensor_scalar(out=tg2[dd][:], in0=tgg[dd][:],
                                            scalar1=2.0, scalar2=-1.0,
                                            op0=ALU.mult, op1=ALU.add)
                    hdst = (hv_ap(dd, u) if u >= W
                            else hscr[dd][:].rearrange("p (c r) -> p c r", c=2))
                    nc.vector.tensor_tensor(
                        out=hdst,
                        in0=sgo[dd][:, 4 * S:6 * S].rearrange("p (c r) -> p c r", c=2),
                        in1=tg2[dd][:].rearrange("p (c r) -> p c r", c=2), op=ALU.mult)
                    nc.vector.tensor_copy(
                        hbf[dd][:].rearrange("p (c r) -> p c r", c=2), hdst)

        # ---------- P7: feats ----------
        with tc.tile_pool(name="psD", bufs=2, space="PSUM") as psD, \
             tc.tile_pool(name="sbD", bufs=2) as sbD:
            fsb = sb.tile([16, FPAD + 512], F32)
            nc.vector.memset(fsb[:], 0.0)
            f_ps = psD.tile([16, 448], F32, space="PSUM", tag="fps")
            first = True
            for dd in range(2):
                for hc in range(2):
                    wt = h2tT[:, (dd * 2 + hc) * 16:(dd * 2 + hc) * 16 + 16]
                    nc.tensor.matmul(f_ps[:, :BF], wt, hval[dd][:, hc * BF:(hc + 1) * BF],
                                     start=first, stop=(dd == 1 and hc == 1))
                    first = False
            nc.vector.tensor_copy(fsb[:, FPAD:FPAD + BF], f_ps[:, :BF])
            fz = sb.tile([128, K], F32)
            nc.vector.memset(fz[:], 0.0)
            nc.sync.dma_start(_dap(feats_d, 0, [[K, FPAD], [1, K]]), fz[:FPAD, :])
            nc.sync.dma_start(_dap(feats_d, (FPAD + BF) * K, [[K, FD - FPAD - BF], [1, K]]),
                              fz[:FD - FPAD - BF, :])
            for g in range(4):
                tp2 = psD.tile([128, 16], F32, space="PSUM", tag="ftp")
                nc.tensor.transpose(tp2[:, :16], fsb[:16, FPAD + g * 128:FPAD + (g + 1) * 128],
                                    ident[:16, :16])
                ftile = sbD.tile([128, K], F32, tag="ftile")
                nc.vector.tensor_copy(ftile[:], tp2[:, :K])
                nc.sync.dma_start(
                    _dap(feats_d, (FPAD + g * 128) * K, [[K, 128], [1, K]]), ftile[:])

        # ---------- P8: viterbi feats views + patch ----------
        NV = NSUPV + 1  # 60
        with tc.tile_pool(name="sbE", bufs=1) as sbE, \
             tc.tile_pool(name="psE", bufs=2, space="PSUM") as psE:
            fva = sbE.tile([128, NV * K], F32, tag="fva")
            fvb = sbE.tile([128, NV * K], F32, tag="fvb")
            vscr = sbE.tile([128, NV * K], F32, tag="vscr")
            vmsk = sbE.tile([128, NV * K], F32, tag="vmsk")
            vpat = sbE.tile([128, NV * K], F32, tag="vpat")
            AOFF, ASTEP, AUST = FPAD * K, B2V * K, K
            BOFF, BSTEP, BUST = (FPAD + 5) * K, B2V * K, K
            for vt, off, pstep, ustep in ((fva, AOFF, ASTEP, AUST),
                                          (fvb, BOFF, BSTEP, BUST)):
                nc.sync.dma_start(vscr[:].rearrange("p (u k) -> p u k", k=K),
                                  _dap(feats_d, off, [[pstep, 128], [ustep, NV], [1, K]]))
                nc.sync.dma_start(vmsk[:].rearrange("p (u k) -> p u k", k=K),
                                  _dap(d["fmask"], off, [[pstep, 128], [ustep, NV], [1, K]]))
                nc.sync.dma_start(vpat[:].rearrange("p (u k) -> p u k", k=K),
                                  _dap(d["fpatch"], off, [[pstep, 128], [ustep, NV], [1, K]]))
                nc.vector.tensor_mul(vmsk[:], vscr[:], vmsk[:])
                nc.vector.tensor_add(vt[:], vmsk[:], vpat[:])

            # ---------- P9: alpha/beta scans ----------
            absb = sbE.tile([128, 2 * 3 * K], F32, tag="absb")
            for which, fview, trr in ((0, fva, trrep), (1, fvb, trrepb)):
                fv = sbE.tile([128, K], F32, tag=f"fv{which}", name=f"fv{which}")
                fvR = sbE.tile([128, K], F32, tag=f"fvR{which}", name=f"fvR{which}")
                nc.vector.memset(fv[:], 0.0)
                tmpv = sbE.tile([128, K * K], F32, tag=f"tmpv{which}", name=f"tmpv{which}")
                f3 = fview[:].rearrange("p (u k) -> p u k", k=K)
                for u in range(NV):
                    uf = u if which == 0 else (NV - 1 - u)
                    nc.vector.tensor_tensor(
                        out=tmpv[:].rearrange("p (i j) -> p i j", i=K),
                        in0=fv[:].rearrange("p (o j) -> p o j", o=1).to_broadcast([128, K, K]),
                        in1=trr[:].rearrange("p (i j) -> p i j", i=K), op=ALU.add)
                    nc.vector.tensor_reduce(fvR[:], tmpv[:].rearrange(
                        "p (i j) -> p i j", i=K), axis=AX.X, op=ALU.max)
                    nc.vector.tensor_add(fv[:], fvR[:], f3[:, uf, :])
                    if 56 <= u < 59:
                        slot = which * 3 + ((u - 56) if which == 0 else (58 - u))
                        nc.vector.tensor_copy(absb[:, slot * K:(slot + 1) * K], fv[:])
            zt = sbE.tile([128, K], F32, tag="zt")
            nc.vector.memset(zt[:], 0.0)
            for which, zhi in ((0, FPAD + 56), (1, FPAD + 6)):
                base = which * FD * K
                for z0 in range(0, zhi, 128):
                    zn = min(128, zhi - z0)
                    nc.sync.dma_start(_dap(ab_d, base + z0 * K, [[K, zn], [1, K]]),
                                      zt[:zn, :])
                nc.sync.dma_start(
                    _dap(ab_d, base + (FPAD + BF) * K, [[K, FD - FPAD - BF], [1, K]]),
                    zt[:FD - FPAD - BF, :])
            # alpha rows: n*3 + 56 + s ; beta rows: 390 - 3n - s
            nc.sync.dma_start(
                _dap(ab_d, (FPAD + 56) * K, [[3 * K, 128], [K, 3], [1, K]]),
                absb[:, 0:3 * K].rearrange("p (s k) -> p s k", k=K))
            nc.sync.dma_start(
                _dap(ab_d, FD * K + (FPAD + 6) * K, [[3 * K, 128], [K, 3], [1, K]]),
                absb[:, 3 * K:6 * K].rearrange("p (s k) -> p s k", k=K))

            # ---------- P10: path + score ----------
            def aligned(src, base, shift):
                return _dap(src, base + (FPAD + shift) * K, [[K, 128], [128 * K, 3], [1, K]])

            al_al = sbE.tile([128, 3 * K], F32, tag="alal")
            be_al = sbE.tile([128, 3 * K], F32, tag="beal")
            al_p = sbE.tile([128, 3 * K], F32, tag="alp")
            be_p = sbE.tile([128, 3 * K], F32, tag="bep")
            ft_al = sbE.tile([128, 3 * K], F32, tag="ftal")
            ft_p = sbE.tile([128, 3 * K], F32, tag="ftp2")
            for dst, src, base, sh in ((al_al, ab_d, 0, 0), (be_al, ab_d, FD * K, 0),
                                       (al_p, ab_d, 0, -1), (be_p, ab_d, FD * K, -1),
                                       (ft_al, feats_d, 0, 0), (ft_p, feats_d, 0, -1)):
                nc.sync.dma_start(dst[:].rearrange("p (g k) -> p g k", k=K),
                                  aligned(src, base, sh))
            tot = sbE.tile([128, 3 * K], F32, tag="tot")
            totp = sbE.tile([128, 3 * K], F32, tag="totp")
            gsc = sbE.tile([128, 3 * K], F32, tag="gsc")
            # tot = alpha + (gamma - feats)   (beta scan state is gamma = beta + feats)
            nc.vector.tensor_tensor(out=gsc[:], in0=be_al[:], in1=ft_al[:], op=ALU.subtract)
            nc.vector.tensor_add(tot[:], al_al[:], gsc[:])
            nc.vector.tensor_tensor(out=gsc[:], in0=be_p[:], in1=ft_p[:], op=ALU.subtract)
            nc.vector.tensor_add(totp[:], al_p[:], gsc[:])
            mx = sbE.tile([128, 3], F32, tag="mx")
            mxp = sbE.tile([128, 3], F32, tag="mxp")
            msk = sbE.tile([128, 3 * K], F32, tag="msk")
            mskp = sbE.tile([128, 3 * K], F32, tag="mskp")
            for mm, tt2, mk in ((mx, tot, msk), (mxp, totp, mskp)):
                nc.vector.tensor_reduce(mm[:], tt2[:].rearrange("p (g k) -> p g k", k=K),
                                        axis=AX.X, op=ALU.max)
                nc.vector.tensor_tensor(
                    out=mk[:].rearrange("p (g k) -> p g k", k=K),
                    in0=tt2[:].rearrange("p (g k) -> p g k", k=K),
                    in1=mm[:].rearrange("p (g o) -> p g o", o=1).to_broadcast([128, 3, K]),
                    op=ALU.is_ge)
            iotaK = sbE.tile([128, K], F32, tag="iotaK")
            nc.gpsimd.iota(iotaK[:], pattern=[[1, K]], base=0, channel_multiplier=0,
                           allow_small_or_imprecise_dtypes=True)
            wrk = sbE.tile([128, 3 * K], F32, tag="wrk")
            nc.vector.tensor_scalar(out=wrk[:], in0=iotaK[:].rearrange(
                "p (o k) -> p o k", o=1).to_broadcast([128, 3, K]), scalar1=-1.0,
                scalar2=float(K - 1), op0=ALU.mult, op1=ALU.add)
            wrk2 = sbE.tile([128, 3 * K], F32, tag="wrk2")
            nc.vector.tensor_mul(wrk2[:], wrk[:], msk[:])
            pathf = sbE.tile([128, 3], F32, tag="pathf")
            pathf2 = sbE.tile([128, 3], F32, tag="pathf2")
            nc.vector.tensor_reduce(pathf2[:], wrk2[:].rearrange("p (g k) -> p g k", k=K),
                                    axis=AX.X, op=ALU.max)
            nc.vector.tensor_scalar(out=pathf[:], in0=pathf2[:], scalar1=-1.0,
                                    scalar2=float(K - 1), op0=ALU.mult, op1=ALU.add)
            pathi = sbE.tile([128, 3], I32, tag="pathi")
            nc.vector.tensor_copy(pathi[:], pathf[:])
            # rows r in [57, 313) -> path_o[0:256]
            nc.sync.dma_start(_dap(path_o, 0, [[1, 71]]), pathi[57:128, 0:1])
            nc.sync.dma_start(_dap(path_o, 71, [[1, 128]]), pathi[:, 1:2])
            nc.sync.dma_start(_dap(path_o, 199, [[1, 57]]), pathi[0:57, 2:3])

            # score partials
            sc0 = sbE.tile([128, 3 * K], F32, tag="sc0")
            sc1 = sbE.tile([128, 3 * K], F32, tag="sc1")
            nc.vector.tensor_mul(sc0[:], ft_al[:], msk[:])
            nc.vector.tensor_tensor(
                out=sc1[:].rearrange("p (g k) -> p g k", k=K),
                in0=sc0[:].rearrange("p (g k) -> p g k", k=K),
                in1=vmask1[:].rearrange("p (g o) -> p g o", o=1).to_broadcast([128, 3, K]),
                op=ALU.mult)
            w12 = sbE.tile([128, 3 * K * K], F32, tag="w12")
            nc.vector.tensor_tensor(
                out=w12[:].rearrange("p (g i j) -> p g i j", i=K, j=K),
                in0=trrep[:].rearrange("p (o i j) -> p o i j", o=1, i=K).to_broadcast([128, 3, K, K]),
                in1=mskp[:].rearrange("p (g o j) -> p g o j", o=1, j=K).to_broadcast([128, 3, K, K]), op=ALU.mult)
            wred0 = sbE.tile([128, 3 * K], F32, tag="wred0")
            wred1 = sbE.tile([128, 3 * K], F32, tag="wred1")
            wred = sbE.tile([128, 3 * K], F32, tag="wred")
            nc.vector.tensor_reduce(wred0[:], w12[:].rearrange(
                "p (g i j) -> p g i j", i=K, j=K), axis=AX.X, op=ALU.add)
            nc.vector.tensor_mul(wred1[:], wred0[:], msk[:])
            nc.vector.tensor_tensor(
                out=wred[:].rearrange("p (g k) -> p g k", k=K),
                in0=wred1[:].rearrange("p (g k) -> p g k", k=K),
                in1=vmask2[:].rearrange("p (g o) -> p g o", o=1).to_broadcast([128, 3, K]),
                op=ALU.mult)
            s3 = sbE.tile([128, K], F32, tag="s3")
            nc.vector.tensor_mul(s3[:], trstop[:], msk[:, 2 * K:3 * K])
            acc = sbE.tile([128, 3 * K], F32, tag="acc")
            nc.vector.tensor_add(acc[:], sc1[:], wred[:])
            stot = sbE.tile([128, 1], F32, tag="stot")
            nc.vector.tensor_reduce(stot[:], acc[:], axis=AX.X, op=ALU.add)
            s3r = sbE.tile([128, 1], F32, tag="s3r")
            s3m = sbE.tile([128, 1], F32, tag="s3m")
            stot2 = sbE.tile([128, 1], F32, tag="stot2")
            nc.vector.tensor_reduce(s3r[:], s3[:], axis=AX.X, op=ALU.add)
            nc.vector.tensor_mul(s3m[:], s3r[:], vmask3[:])
            nc.vector.tensor_add(stot2[:], stot[:], s3m[:])
            nc.sync.dma_start(score_o.ap(), stot2[:])

            if debug_outputs:
                nc.sync.dma_start(dbg["feats_o"].ap(), feats_d.ap())
                for c in range(3):
                    nc.sync.dma_start(dbg["emb_o"].ap()[c], embT[c][:])
                for dd2 in range(2):
                    nc.sync.dma_start(dbg["h_o"].ap()[dd2], hval[dd2][:])
                nc.sync.dma_start(dbg["ab_o"].ap(), ab_d.ap())

    nc.compile()
    return nc


# ---------------- host-side prep ----------------
def make_in_maps(inputs):
    inputs = {k: np.asarray(v) for k, v in inputs.items()}
    sentence = inputs["sentence"].astype(np.int64)
    chars = inputs["chars"].astype(np.int64)
    word_emb = np.concatenate(
        [inputs["word_emb"].astype(np.float32), np.zeros((1, E), np.float32)], 0)
    import ml_dtypes

    # shared param layouts
    def lhsT_tiles(w, kdim, kchunks, mchunks):
        # w: [out(gate), in] -> tiles [(kc, mc)] each [128(K), 128(M)]
        out = np.zeros((128, kchunks * mchunks * 128), np.float32)
        for kc in range(kchunks):
            for mc in range(mchunks):
                blk = w[mc * 128:(mc + 1) * 128, kc * 128:(kc + 1) * 128]
                out[:, (kc * mchunks + mc) * 128:(kc * mchunks + mc) * 128 + 128] = blk.T
        return out

    w_ih_ext = {}
    for dd, (wn, bn) in enumerate((("w_ih_f", "b_f"), ("w_ih_b", "b_b"))):
        wext = np.zeros((4 * H, 384), np.float32)
        wext[:, :300] = inputs[wn][:, :300]
        wext[:, 320:345] = inputs[wn][:, 300:325]
        wext[:, 352] = inputs[bn]
        w_ih_ext[dd] = wext
    w_ihT = np.concatenate([lhsT_tiles(w_ih_ext[dd], 384, 3, 8) for dd in range(2)], 1)
    w_hhT = np.concatenate(
        [lhsT_tiles(inputs[wn], 256, 2, 8) for wn in ("w_hh_f", "w_hh_b")], 1
    ).astype(ml_dtypes.bfloat16)
    h2tT = np.zeros((128, 2 * 2 * 16), np.float32)
    for dd in range(2):
        for hc in range(2):
            blk = inputs["h2t_w"][:, dd * 256 + hc * 128: dd * 256 + (hc + 1) * 128]
            h2tT[:, (dd * 2 + hc) * 16:(dd * 2 + hc) * 16 + 12] = blk.T
    cembT = inputs["char_emb"].astype(np.float32).T.copy()           # [25, 85]
    convT = inputs["conv_w"][:, 0, :, :].transpose(2, 1, 0).reshape(CE, 3 * CO).copy()
    convb = inputs["conv_b"].astype(np.float32).reshape(CO, 1)
    trans = inputs["transitions"].astype(np.float32)
    trrep = np.tile(trans.reshape(1, K * K), (128, 1))
    ident = np.eye(128, dtype=np.float32)
    iota85 = np.arange(CV).reshape(CV, 1).astype(np.float32)

    in_maps = []
    for k in range(NC):
        m = {"word_emb": word_emb, "w_ihT": w_ihT, "w_hhT": w_hhT, "h2tT": h2tT,
             "cembT": cembT, "convT": convT, "convb": convb, "trrep": trrep,
             "ident": ident, "iota85": iota85}
        # gather indices + char grid
        widx = np.full((TEXT, 1), V, np.int32)
        charsb = np.full((TEXT, LP), PADCHAR, np.float32)
        for pr in range(TEXT):
            t = pr2t(k, pr)
            if 0 <= t < T:
                widx[pr, 0] = sentence[t]
                charsb[pr, 2:2 + L] = chars[t]
        m["widx"] = widx
        m["charsb"] = charsb.reshape(-1).astype(ml_dtypes.bfloat16)
        # beta transitions: transposed; core 7 masked for terminal STOP selection
        trb = trans.T.copy()
        if k == NC - 1:
            trb[START, STOP] += NEG
            trb[STOP, STOP] += NEG
        m["trrepb"] = np.tile(trb.reshape(1, K * K), (128, 1))
        # feats mask/patch
        fmask = np.ones((FD, K), np.float32)
        fpatch = np.zeros((FD, K), np.float32)
        if k == 0:
            fmask[FPAD + 56] = 0.0
            fpatch[FPAD + 56] = NEG
            fpatch[FPAD + 56, START] = 0.0
        if k == NC - 1:
            r_T = (T - row2t(k, 0))  # feats row for t == T
            fmask[FPAD + r_T] = 0.0
            fpatch[FPAD + r_T] = NEG
            fpatch[FPAD + r_T, STOP] = 0.0
        m["fmask"] = fmask
        m["fpatch"] = fpatch
        m["trstop"] = (np.tile(trans[STOP].reshape(1, K), (128, 1))
                       if k == NC - 1 else np.zeros((128, K), np.float32))
        vm1 = np.zeros((128, 3), np.float32)
        vm2 = np.zeros((128, 3), np.float32)
        for g in range(3):
            for p in range(128):
                r = g * 128 + p
                if 57 <= r < 313:
                    vm1[p, g] = 1.0
                    vm2[p, g] = 1.0
        m["vmask1"] = vm1
        m["vmask2"] = vm2
        vm3 = np.zeros((128, 1), np.float32)
        if k == NC - 1:
            vm3[56, 0] = 1.0
        m["vmask3"] = vm3
        in_maps.append(m)
    return in_maps


_NC_CACHE = {}


def kernel(**inputs):
    from concourse import bass_utils
    key = "main"
    if key not in _NC_CACHE:
        _NC_CACHE[key] = build_nc(debug_outputs=False)
    nc = _NC_CACHE[key]
    in_maps = make_in_maps(inputs)
    res = bass_utils.run_bass_kernel_spmd(nc, in_maps, core_ids=list(range(NC)))
    path = np.concatenate([res.results[k]["path_o"] for k in range(NC)]).astype(np.int32)
    score = np.float32(sum(np.float32(res.results[k]["score_o"].sum()) for k in range(NC)))
    return score, path


# revision 15
# speedup vs baseline: 1.1256x; 1.1256x over previous
"""BiLSTM-CRF Trainium2 kernel: 8-core SPMD, chunk-parallel LSTM + Viterbi.

Strategy (validated numerically against the reference in float32):
- Core k owns output slice [k*256, (k+1)*256) of the T=2048 sequence.
- The LSTM recurrence is chunk-parallelized: per core, per direction, 32
  streams each process a 14-step chunk preceded by a 40-step warmup from zero
  state (state influence decays ~4x/step, so the warmup converges to f32
  noise). Streams are batched in the matmul free dim, so one "superstep" does
  16 bf16 weight-tile matmuls for all 32 streams at once.
- Viterbi alpha (forward) and beta (backward) max-plus scans are
  chunk-parallelized the same way (128 streams x 3 steps, 56-step coalescence
  warmup); path[t] = argmax_i(alpha_t[i] + beta_t[i]).
- The score is recomputed from the decoded path (sum of edge scores) because
  chunked scans lose the global additive constant; per-core partials are
  summed on the host (part of unsharding).
- Sequence edges are handled with data only (all 8 cores run one program):
  virtual rows outside [0,T) use a zero word-embedding row and a pad char id;
  the alpha START init and the beta STOP init are injected via patched feats
  rows and a masked replicated transition matrix on the affected cores.

kernel(**inputs) -> (score, path) matching reference.reference().
"""
import numpy as np
from contextlib import ExitStack

import concourse.bass as bass
import concourse.tile as tile
from concourse import bacc, mybir

F32 = mybir.dt.float32
F32R = mybir.dt.float32r
BF16 = mybir.dt.bfloat16
I32 = mybir.dt.int32
AF = mybir.ActivationFunctionType
ALU = mybir.AluOpType
AX = mybir.AxisListType

# ---------------- geometry ----------------
T, K, H, E, CO, CE, CV, L = 2048, 12, 256, 300, 25, 25, 85, 20
V = 100000
START, STOP = 10, 11
NEG = -10000.0
NC = 8
B = T // NC          # 256

W = 32               # LSTM warmup steps
WV = 48              # Viterbi warmup steps
S = 64               # LSTM streams per direction
BF = 448             # feats rows per core; feats row r <-> t = k*B - 57 + r
B2 = BF // S         # 14
NSUP = B2 + W        # 54 LSTM supersteps
PR = BF + 2 * W      # 528 precomp rows; precomp row pr = feats row + W
TEXT = 640           # padded emb/precomp columns (5*128)
LP = 24              # padded word length
NCH = TEXT * LP
B2V = 3
NSUPV = WV + B2V     # 59
FPAD = 80            # junk rows below feats row 0 in DRAM buffers
FD = 608             # DRAM rows in feats/ab buffers (FPAD + 512 + 16)
PADCHAR = 200.0
RV0 = WV + 1         # feats row of t = k*B (first output row)
BROW = BF - (NSUPV + 1) - 383   # beta view base row
BSROW = BF - 386 - WV           # beta stored base row


def row2t(k, r):     # feats row -> global t
    return k * B - (WV + 1) + r


def pr2t(k, pr):     # precomp/emb column -> global t
    return k * B - (WV + 1 + W) + pr


def _dap(dram, offset, pairs):
    """Raw strided AP over a DRAM tensor (element units)."""
    return bass.AP(dram, offset, [list(p) for p in pairs])


def build_nc(debug_outputs=False):
    nc = bacc.Bacc("TRN2", target_bir_lowering=False, debug=False, num_devices=NC)
    d = {}

    def ein(n, sh, dt):
        d[n] = nc.dram_tensor(n, sh, dt, kind="ExternalInput")

    ein("word_emb", [V + 1, E], F32)
    ein("widx", [TEXT, 1], I32)
    ein("charsb", [NCH], BF16)
    ein("iota85", [CV, 1], F32)
    ein("ident", [128, 128], F32)
    ein("w_ihT", [128, 2 * 3 * 8 * 128], F32)
    ein("w_hhT", [128, 2 * 2 * 8 * 128], BF16)
    ein("h2tT", [128, 2 * 2 * 16], F32)
    ein("cembT", [CE, CV], F32)
    ein("convT", [CE, 3 * CO], F32)
    ein("convb", [CO, 1], F32)
    ein("trrep", [128, K * K], F32)
    ein("trrepb", [128, K * K], F32)
    ein("fmask", [FD, K], F32)
    ein("fpatch", [FD, K], F32)
    ein("trstop", [128, K], F32)
    ein("vmask1", [128, 3], F32)
    ein("vmask2", [128, 3], F32)
    ein("vmask3", [128, 1], F32)

    path_o = nc.dram_tensor("path_o", [B], I32, kind="ExternalOutput")
    score_o = nc.dram_tensor("score_o", [128, 1], F32, kind="ExternalOutput")
    dbg = {}
    if debug_outputs:
        dbg["feats_o"] = nc.dram_tensor("feats_o", [FD, K], F32, kind="ExternalOutput")
        dbg["emb_o"] = nc.dram_tensor("emb_o", [3, 128, TEXT], F32, kind="ExternalOutput")
        dbg["h_o"] = nc.dram_tensor("h_o", [2, 128, 2 * BF], F32, kind="ExternalOutput")
        dbg["ab_o"] = nc.dram_tensor("ab_o", [2, FD, K], F32, kind="ExternalOutput")

    feats_d = nc.dram_tensor("feats_d", [FD, K], F32, kind="Internal")
    ab_d = nc.dram_tensor("ab_d", [2, FD, K], F32, kind="Internal")

    with tile.TileContext(nc) as tc, ExitStack() as ctx:
        sb = ctx.enter_context(tc.tile_pool(name="sb", bufs=1))

        # ---------- P0: params ----------
        w_ihT = sb.tile([128, 2 * 3 * 8 * 128], F32R)
        nc.gpsimd.dma_start(w_ihT[:], d["w_ihT"].ap())
        w_hhT = sb.tile([128, 2 * 2 * 8 * 128], BF16)
        nc.sync.dma_start(w_hhT[:], d["w_hhT"].ap())
        h2tT = sb.tile([128, 2 * 2 * 16], F32)
        nc.sync.dma_start(h2tT[:], d["h2tT"].ap())
        ident = sb.tile([128, 128], F32)
        nc.sync.dma_start(ident[:], d["ident"].ap())
        trrep = sb.tile([128, K * K], F32)
        nc.sync.dma_start(trrep[:], d["trrep"].ap())
        trrepb = sb.tile([128, K * K], F32)
        nc.sync.dma_start(trrepb[:], d["trrepb"].ap())
        trstop = sb.tile([128, K], F32)
        nc.sync.dma_start(trstop[:], d["trstop"].ap())
        vmask1 = sb.tile([128, 3], F32)
        nc.sync.dma_start(vmask1[:], d["vmask1"].ap())
        vmask2 = sb.tile([128, 3], F32)
        nc.sync.dma_start(vmask2[:], d["vmask2"].ap())
        vmask3 = sb.tile([128, 1], F32)
        nc.sync.dma_start(vmask3[:], d["vmask3"].ap())

        embT = [sb.tile([128, TEXT], F32, tag=f"embT{c}", name=f"embT{c}") for c in range(3)]
        embTr = [sb.tile([128, TEXT], F32R, tag=f"embTr{c}", name=f"embTr{c}") for c in range(3)]
        precompT = [sb.tile([128, 8 * TEXT], F32, tag=f"pre{dd}", name=f"pre{dd}") for dd in range(2)]
        hval = [sb.tile([128, 2 * BF], F32, tag=f"hval{dd}", name=f"hval{dd}") for dd in range(2)]

        # ---------- P1-P4: embeddings + charCNN ----------
        with tc.tile_pool(name="psA", bufs=2, space="PSUM") as psA, \
             tc.tile_pool(name="sbA", bufs=1) as sbA:
            iota85 = sbA.tile([CV, 1], F32, tag="iota85")
            nc.sync.dma_start(iota85[:], d["iota85"].ap())
            cembT = sbA.tile([CE, CV], F32, tag="cembT")
            nc.sync.dma_start(cembT[:], d["cembT"].ap())
            convT = sbA.tile([CE, 3 * CO], F32, tag="convT")
            nc.sync.dma_start(convT[:], d["convT"].ap())
            convb = sbA.tile([128, 1], F32, tag="convb")
            nc.sync.dma_start(convb[64:64 + CO, :], d["convb"].ap())

            ctab_ps = psA.tile([CV, 3 * CO], F32, space="PSUM", tag="ctab")
            nc.tensor.matmul(ctab_ps[:], cembT[:], convT[:], start=True, stop=True)
            ctab = sbA.tile([CV, 3 * CO], BF16, tag="ctabsb")
            nc.vector.tensor_copy(ctab[:], ctab_ps[:])

            onehot = sbA.tile([CV, NCH], BF16, tag="onehot")
            CHCH = 3840
            for c0 in range(0, NCH, CHCH):
                chbc = sbA.tile([CV, CHCH], BF16, tag="chb", bufs=2, name=f"chb{c0}")
                nc.sync.dma_start(chbc[:], _dap(d["charsb"], c0, [[0, CV], [1, CHCH]]))
                nc.vector.tensor_scalar(out=onehot[:, c0:c0 + CHCH], in0=chbc[:],
                                        scalar1=iota85[:], scalar2=None,
                                        op0=ALU.is_equal)

            for c in range(3):
                nc.vector.memset(embT[c][:], 0.0)

            widx = sbA.tile([128, 5], I32, tag="widx")
            nc.sync.dma_start(widx[:], _dap(d["widx"], 0, [[1, 128], [128, 5]]))
            for g in range(5):
                wrows = sbA.tile([128, 304], F32, tag="wrows", bufs=2, name=f"wrows{g}")
                nc.gpsimd.indirect_dma_start(
                    out=wrows[:, :300], out_offset=None, in_=d["word_emb"].ap(),
                    in_offset=bass.IndirectOffsetOnAxis(ap=widx[:, g:g + 1], axis=0))
                for c in range(3):
                    dim0 = c * 128
                    ncols = min(128, 300 - dim0)
                    tp = psA.tile([128, 128], F32, space="PSUM", tag="tp")
                    nc.tensor.transpose(tp[:ncols, :], wrows[:, dim0:dim0 + ncols],
                                        ident[:])
                    nc.vector.tensor_copy(embT[c][:ncols, g * 128:(g + 1) * 128],
                                          tp[:ncols, :128])
            nc.vector.memset(embT[2][96:97, :], 1.0)

            # charCNN: co[d,(t,j)] = sum_k Tk-matmul over windowed onehot
            TCW = 23
            oh3 = onehot[:].rearrange("c (t l) -> c t l", l=LP)
            nsl = (TEXT + TCW - 1) // TCW
            for sl in range(nsl):
                t0 = sl * TCW
                tn = min(TCW, TEXT - t0)
                co_ps = psA.tile([128, TCW * (L + 2)], F32, space="PSUM", tag="co")
                co3 = co_ps[64:64 + CO, :tn * (L + 2)].rearrange("d (t j) -> d t j", j=L + 2)
                for k3 in range(3):
                    nc.tensor.matmul(co3, ctab[:, k3 * CO:(k3 + 1) * CO],
                                     oh3[:, t0:t0 + tn, k3:k3 + L + 2],
                                     start=(k3 == 0), stop=(k3 == 2))
                cf = sbA.tile([128, TCW], F32, tag="cf", bufs=2, name=f"cf{sl}")
                nc.vector.tensor_reduce(cf[64:64 + CO, :tn], co3, axis=AX.X, op=ALU.max)
                nc.vector.tensor_scalar(out=embT[2][64:64 + CO, t0:t0 + tn],
                                        in0=cf[64:64 + CO, :tn],
                                        scalar1=convb[64:64 + CO, :],
                                        scalar2=None, op0=ALU.add)

            for c in range(3):
                nc.vector.tensor_copy(embTr[c][:], embT[c][:])

        # ---------- P5: input matmuls ----------
        with tc.tile_pool(name="psB", bufs=2, space="PSUM") as psB:
            for dd in range(2):
                for mc in range(8):
                    pc_ps = psB.tile([128, TEXT], F32, space="PSUM", tag="pcps")
                    for n0, nn in ((0, 512), (512, 128)):
                        for kc in range(3):
                            w0 = ((dd * 3 + kc) * 8 + mc) * 128
                            nc.tensor.matmul(pc_ps[:, n0:n0 + nn],
                                             w_ihT[:, w0:w0 + 128],
                                             embTr[kc][:, n0:n0 + nn],
                                             start=(kc == 0), stop=(kc == 2))
                    nc.scalar.activation(precompT[dd][:, mc * TEXT:(mc + 1) * TEXT],
                                         pc_ps[:], AF.Copy)

        # ---------- P6: LSTM supersteps ----------
        def pr_ap(dd, u):
            t3 = precompT[dd][:].rearrange("p (c r) -> p c r", c=8)
            if dd == 0:
                return t3[:, :, u:u + (S - 1) * B2 + 1:B2]
            base = PR - 1 - u
            return t3[:, :, base - (S - 1) * B2:base + 1:B2]

        def hv_ap(dd, u):
            t3 = hval[dd][:].rearrange("p (c r) -> p c r", c=2)
            if dd == 0:
                r0 = u - W
                return t3[:, :, r0:r0 + (S - 1) * B2 + 1:B2]
            base = BF - 1 - (u - W)
            return t3[:, :, base - (S - 1) * B2:base + 1:B2]

        with tc.tile_pool(name="psC", bufs=2, space="PSUM") as psC, \
             tc.tile_pool(name="sbC", bufs=1) as sbC:
            hscr = [sbC.tile([128, 2 * S], F32, tag=f"hscr{dd}", name=f"hscr{dd}") for dd in range(2)]
            hbf = [sbC.tile([128, 2 * S], BF16, tag=f"hbf{dd}", name=f"hbf{dd}") for dd in range(2)]
            cstA = [sbC.tile([128, 2 * S], F32, tag=f"cstA{dd}", name=f"cstA{dd}") for dd in range(2)]
            cstB = [sbC.tile([128, 2 * S], F32, tag=f"cstB{dd}", name=f"cstB{dd}") for dd in range(2)]
            gsb = [sbC.tile([128, 8 * S], F32, tag=f"gsb{dd}", name=f"gsb{dd}") for dd in range(2)]
            sgo = [sbC.tile([128, 6 * S], F32, tag=f"sgo{dd}", name=f"sgo{dd}") for dd in range(2)]
            tgg = [sbC.tile([128, 2 * S], F32, tag=f"tgg{dd}", name=f"tgg{dd}") for dd in range(2)]
            tg2 = [sbC.tile([128, 2 * S], F32, tag=f"tg2{dd}", name=f"tg2{dd}") for dd in range(2)]
            tm1 = [sbC.tile([128, 2 * S], F32, tag=f"tm1{dd}", name=f"tm1{dd}") for dd in range(2)]
            for dd in range(2):
                nc.vector.memset(hbf[dd][:], 0.0)
                nc.vector.memset(cstA[dd][:], 0.0)
                nc.vector.memset(hval[dd][:], 0.0)

            for u in range(NSUP):
                for dd in range(2):
                    gates = psC.tile([128, 8 * S], F32, space="PSUM", tag=f"g{dd}")
                    for mc in range(8):
                        for kc in range(2):
                            w0 = ((dd * 2 + kc) * 8 + mc) * 128
                            nc.tensor.matmul(gates[:, mc * S:(mc + 1) * S],
                                             w_hhT[:, w0:w0 + 128],
                                             hbf[dd][:, kc * S:(kc + 1) * S],
                                             start=(kc == 0), stop=(kc == 1))
                    g = gsb[dd]
                    nc.vector.tensor_tensor(
                        out=g[:].rearrange("p (c r) -> p c r", c=8),
                        in0=gates[:].rearrange("p (c r) -> p c r", c=8),
                        in1=pr_ap(dd, u), op=ALU.add)
                    nc.scalar.activation(sgo[dd][:], g[:, :6 * S], AF.Sigmoid)
                    nc.scalar.activation(tgg[dd][:], g[:, 6 * S:8 * S], AF.Sigmoid,
                                         scale=2.0)
                    nc.vector.tensor_scalar(out=tg2[dd][:], in0=tgg[dd][:],
                                            scalar1=2.0, scalar2=-1.0,
                                            op0=ALU.mult, op1=ALU.add)
                    nc.vector.tensor_mul(tm1[dd][:], sgo[dd][:, :2 * S], tg2[dd][:])
                    nc.vector.tensor_mul(cstB[dd][:], sgo[dd][:, 2 * S:4 * S],
                                         cstA[dd][:])
                    nc.vector.tensor_add(cstA[dd][:], cstB[dd][:], tm1[dd][:])
                    nc.scalar.activation(tgg[dd][:], cstA[dd][:], AF.Sigmoid, scale=2.0)
                    nc.vector.tensor_scalar(out=tg2[dd][:], in0=tgg[dd][:],
                                            scalar1=2.0, scalar2=-1.0,
                                            op0=ALU.mult, op1=ALU.add)
                    hdst = (hv_ap(dd, u) if u >= W
                            else hscr[dd][:].rearrange("p (c r) -> p c r", c=2))
                    nc.vector.tensor_tensor(
                        out=hdst,
                        in0=sgo[dd][:, 4 * S:6 * S].rearrange("p (c r) -> p c r", c=2),
                        in1=tg2[dd][:].rearrange("p (c r) -> p c r", c=2), op=ALU.mult)
                    nc.vector.tensor_copy(
                        hbf[dd][:].rearrange("p (c r) -> p c r", c=2), hdst)

        # ---------- P7: feats ----------
        with tc.tile_pool(name="psD", bufs=2, space="PSUM") as psD, \
             tc.tile_pool(name="sbD", bufs=2) as sbD:
            fsb = sb.tile([16, FPAD + 512], F32)
            nc.vector.memset(fsb[:], 0.0)
            f_ps = psD.tile([16, 448], F32, space="PSUM", tag="fps")
            first = True
            for dd in range(2):
                for hc in range(2):
                    wt = h2tT[:, (dd * 2 + hc) * 16:(dd * 2 + hc) * 16 + 16]
                    nc.tensor.matmul(f_ps[:, :BF], wt, hval[dd][:, hc * BF:(hc + 1) * BF],
                                     start=first, stop=(dd == 1 and hc == 1))
                    first = False
            nc.vector.tensor_copy(fsb[:, FPAD:FPAD + BF], f_ps[:, :BF])
            fz = sb.tile([128, K], F32)
            nc.vector.memset(fz[:], 0.0)
            nc.sync.dma_start(_dap(feats_d, 0, [[K, FPAD], [1, K]]), fz[:FPAD, :])
            nc.sync.dma_start(_dap(feats_d, (FPAD + BF) * K, [[K, FD - FPAD - BF], [1, K]]),
                              fz[:FD - FPAD - BF, :])
            for g in range(4):
                tp2 = psD.tile([128, 16], F32, space="PSUM", tag="ftp")
                nc.tensor.transpose(tp2[:, :16], fsb[:16, FPAD + g * 128:FPAD + (g + 1) * 128],
                                    ident[:16, :16])
                ftile = sbD.tile([128, K], F32, tag="ftile")
                nc.vector.tensor_copy(ftile[:], tp2[:, :K])
                nc.sync.dma_start(
                    _dap(feats_d, (FPAD + g * 128) * K, [[K, 128], [1, K]]), ftile[:])

        # ---------- P8: viterbi feats views + patch ----------
        NV = NSUPV + 1  # 60
        with tc.tile_pool(name="sbE", bufs=1) as sbE, \
             tc.tile_pool(name="psE", bufs=2, space="PSUM") as psE:
            fva = sbE.tile([128, NV * K], F32, tag="fva")
            fvb = sbE.tile([128, NV * K], F32, tag="fvb")
            vscr = sbE.tile([128, NV * K], F32, tag="vscr")
            vmsk = sbE.tile([128, NV * K], F32, tag="vmsk")
            vpat = sbE.tile([128, NV * K], F32, tag="vpat")
            AOFF, ASTEP, AUST = FPAD * K, B2V * K, K
            BOFF, BSTEP, BUST = (FPAD + BROW) * K, B2V * K, K
            for vt, off, pstep, ustep in ((fva, AOFF, ASTEP, AUST),
                                          (fvb, BOFF, BSTEP, BUST)):
                nc.sync.dma_start(vscr[:].rearrange("p (u k) -> p u k", k=K),
                                  _dap(feats_d, off, [[pstep, 128], [ustep, NV], [1, K]]))
                nc.sync.dma_start(vmsk[:].rearrange("p (u k) -> p u k", k=K),
                                  _dap(d["fmask"], off, [[pstep, 128], [ustep, NV], [1, K]]))
                nc.sync.dma_start(vpat[:].rearrange("p (u k) -> p u k", k=K),
                                  _dap(d["fpatch"], off, [[pstep, 128], [ustep, NV], [1, K]]))
                nc.vector.tensor_mul(vmsk[:], vscr[:], vmsk[:])
                nc.vector.tensor_add(vt[:], vmsk[:], vpat[:])

            # ---------- P9: alpha/beta scans ----------
            absb = sbE.tile([128, 2 * 3 * K], F32, tag="absb")
            for which, fview, trr in ((0, fva, trrep), (1, fvb, trrepb)):
                fv = sbE.tile([128, K], F32, tag=f"fv{which}", name=f"fv{which}")
                fvR = sbE.tile([128, K], F32, tag=f"fvR{which}", name=f"fvR{which}")
                nc.vector.memset(fv[:], 0.0)
                tmpv = sbE.tile([128, K * K], F32, tag=f"tmpv{which}", name=f"tmpv{which}")
                f3 = fview[:].rearrange("p (u k) -> p u k", k=K)
                for u in range(NV):
                    uf = u if which == 0 else (NV - 1 - u)
                    nc.vector.tensor_tensor(
                        out=tmpv[:].rearrange("p (i j) -> p i j", i=K),
                        in0=fv[:].rearrange("p (o j) -> p o j", o=1).to_broadcast([128, K, K]),
                        in1=trr[:].rearrange("p (i j) -> p i j", i=K), op=ALU.add)
                    nc.vector.tensor_reduce(fvR[:], tmpv[:].rearrange(
                        "p (i j) -> p i j", i=K), axis=AX.X, op=ALU.max)
                    nc.vector.tensor_add(fv[:], fvR[:], f3[:, uf, :])
                    if WV <= u < WV + 3:
                        slot = which * 3 + ((u - WV) if which == 0 else (WV + 2 - u))
                        nc.vector.tensor_copy(absb[:, slot * K:(slot + 1) * K], fv[:])
            zt = sbE.tile([128, K], F32, tag="zt")
            nc.vector.memset(zt[:], 0.0)
            for which, zhi in ((0, FPAD + WV), (1, FPAD + BSROW)):
                base = which * FD * K
                for z0 in range(0, zhi, 128):
                    zn = min(128, zhi - z0)
                    nc.sync.dma_start(_dap(ab_d, base + z0 * K, [[K, zn], [1, K]]),
                                      zt[:zn, :])
                nc.sync.dma_start(
                    _dap(ab_d, base + (FPAD + BF) * K, [[K, FD - FPAD - BF], [1, K]]),
                    zt[:FD - FPAD - BF, :])
            # alpha rows: n*3 + 56 + s ; beta rows: 390 - 3n - s
            nc.sync.dma_start(
                _dap(ab_d, (FPAD + WV) * K, [[3 * K, 128], [K, 3], [1, K]]),
                absb[:, 0:3 * K].rearrange("p (s k) -> p s k", k=K))
            nc.sync.dma_start(
                _dap(ab_d, FD * K + (FPAD + BSROW) * K, [[3 * K, 128], [K, 3], [1, K]]),
                absb[:, 3 * K:6 * K].rearrange("p (s k) -> p s k", k=K))

            # ---------- P10: path + score ----------
            def aligned(src, base, shift):
                return _dap(src, base + (FPAD + shift) * K, [[K, 128], [128 * K, 3], [1, K]])

            al_al = sbE.tile([128, 3 * K], F32, tag="alal")
            be_al = sbE.tile([128, 3 * K], F32, tag="beal")
            al_p = sbE.tile([128, 3 * K], F32, tag="alp")
            be_p = sbE.tile([128, 3 * K], F32, tag="bep")
            ft_al = sbE.tile([128, 3 * K], F32, tag="ftal")
            ft_p = sbE.tile([128, 3 * K], F32, tag="ftp2")
            for dst, src, base, sh in ((al_al, ab_d, 0, 0), (be_al, ab_d, FD * K, 0),
                                       (al_p, ab_d, 0, -1), (be_p, ab_d, FD * K, -1),
                                       (ft_al, feats_d, 0, 0), (ft_p, feats_d, 0, -1)):
                nc.sync.dma_start(dst[:].rearrange("p (g k) -> p g k", k=K),
                                  aligned(src, base, sh))
            tot = sbE.tile([128, 3 * K], F32, tag="tot")
            totp = sbE.tile([128, 3 * K], F32, tag="totp")
            gsc = sbE.tile([128, 3 * K], F32, tag="gsc")
            # tot = alpha + (gamma - feats)   (beta scan state is gamma = beta + feats)
            nc.vector.tensor_tensor(out=gsc[:], in0=be_al[:], in1=ft_al[:], op=ALU.subtract)
            nc.vector.tensor_add(tot[:], al_al[:], gsc[:])
            nc.vector.tensor_tensor(out=gsc[:], in0=be_p[:], in1=ft_p[:], op=ALU.subtract)
            nc.vector.tensor_add(totp[:], al_p[:], gsc[:])
            mx = sbE.tile([128, 3], F32, tag="mx")
            mxp = sbE.tile([128, 3], F32, tag="mxp")
            msk = sbE.tile([128, 3 * K], F32, tag="msk")
            mskp = sbE.tile([128, 3 * K], F32, tag="mskp")
            for mm, tt2, mk in ((mx, tot, msk), (mxp, totp, mskp)):
                nc.vector.tensor_reduce(mm[:], tt2[:].rearrange("p (g k) -> p g k", k=K),
                                        axis=AX.X, op=ALU.max)
                nc.vector.tensor_tensor(
                    out=mk[:].rearrange("p (g k) -> p g k", k=K),
                    in0=tt2[:].rearrange("p (g k) -> p g k", k=K),
                    in1=mm[:].rearrange("p (g o) -> p g o", o=1).to_broadcast([128, 3, K]),
                    op=ALU.is_ge)
            iotaK = sbE.tile([128, K], F32, tag="iotaK")
            nc.gpsimd.iota(iotaK[:], pattern=[[1, K]], base=0, channel_multiplier=0,
                           allow_small_or_imprecise_dtypes=True)
            wrk = sbE.tile([128, 3 * K], F32, tag="wrk")
            nc.vector.tensor_scalar(out=wrk[:], in0=iotaK[:].rearrange(
                "p (o k) -> p o k", o=1).to_broadcast([128, 3, K]), scalar1=-1.0,
                scalar2=float(K - 1), op0=ALU.mult, op1=ALU.add)
            wrk2 = sbE.tile([128, 3 * K], F32, tag="wrk2")
            nc.vector.tensor_mul(wrk2[:], wrk[:], msk[:])
            pathf = sbE.tile([128, 3], F32, tag="pathf")
            pathf2 = sbE.tile([128, 3], F32, tag="pathf2")
            nc.vector.tensor_reduce(pathf2[:], wrk2[:].rearrange("p (g k) -> p g k", k=K),
                                    axis=AX.X, op=ALU.max)
            nc.vector.tensor_scalar(out=pathf[:], in0=pathf2[:], scalar1=-1.0,
                                    scalar2=float(K - 1), op0=ALU.mult, op1=ALU.add)
            pathi = sbE.tile([128, 3], I32, tag="pathi")
            nc.vector.tensor_copy(pathi[:], pathf[:])
            # rows r in [57, 313) -> path_o[0:256]
            n0 = 128 - RV0
            nc.sync.dma_start(_dap(path_o, 0, [[1, n0]]), pathi[RV0:128, 0:1])
            nc.sync.dma_start(_dap(path_o, n0, [[1, 128]]), pathi[:, 1:2])
            nc.sync.dma_start(_dap(path_o, n0 + 128, [[1, RV0]]), pathi[0:RV0, 2:3])

            # score partials
            sc0 = sbE.tile([128, 3 * K], F32, tag="sc0")
            sc1 = sbE.tile([128, 3 * K], F32, tag="sc1")
            nc.vector.tensor_mul(sc0[:], ft_al[:], msk[:])
            nc.vector.tensor_tensor(
                out=sc1[:].rearrange("p (g k) -> p g k", k=K),
                in0=sc0[:].rearrange("p (g k) -> p g k", k=K),
                in1=vmask1[:].rearrange("p (g o) -> p g o", o=1).to_broadcast([128, 3, K]),
                op=ALU.mult)
            w12 = sbE.tile([128, 3 * K * K], F32, tag="w12")
            nc.vector.tensor_tensor(
                out=w12[:].rearrange("p (g i j) -> p g i j", i=K, j=K),
                in0=trrep[:].rearrange("p (o i j) -> p o i j", o=1, i=K).to_broadcast([128, 3, K, K]),
                in1=mskp[:].rearrange("p (g o j) -> p g o j", o=1, j=K).to_broadcast([128, 3, K, K]), op=ALU.mult)
            wred0 = sbE.tile([128, 3 * K], F32, tag="wred0")
            wred1 = sbE.tile([128, 3 * K], F32, tag="wred1")
            wred = sbE.tile([128, 3 * K], F32, tag="wred")
            nc.vector.tensor_reduce(wred0[:], w12[:].rearrange(
                "p (g i j) -> p g i j", i=K, j=K), axis=AX.X, op=ALU.add)
            nc.vector.tensor_mul(wred1[:], wred0[:], msk[:])
            nc.vector.tensor_tensor(
                out=wred[:].rearrange("p (g k) -> p g k", k=K),
                in0=wred1[:].rearrange("p (g k) -> p g k", k=K),
                in1=vmask2[:].rearrange("p (g o) -> p g o", o=1).to_broadcast([128, 3, K]),
                op=ALU.mult)
            s3 = sbE.tile([128, K], F32, tag="s3")
            nc.vector.tensor_mul(s3[:], trstop[:], msk[:, 2 * K:3 * K])
            acc = sbE.tile([128, 3 * K], F32, tag="acc")
            nc.vector.tensor_add(acc[:], sc1[:], wred[:])
            stot = sbE.tile([128, 1], F32, tag="stot")
            nc.vector.tensor_reduce(stot[:], acc[:], axis=AX.X, op=ALU.add)
            s3r = sbE.tile([128, 1], F32, tag="s3r")
            s3m = sbE.tile([128, 1], F32, tag="s3m")
            stot2 = sbE.tile([128, 1], F32, tag="stot2")
            nc.vector.tensor_reduce(s3r[:], s3[:], axis=AX.X, op=ALU.add)
            nc.vector.tensor_mul(s3m[:], s3r[:], vmask3[:])
            nc.vector.tensor_add(stot2[:], stot[:], s3m[:])
            nc.sync.dma_start(score_o.ap(), stot2[:])

            if debug_outputs:
                nc.sync.dma_start(dbg["feats_o"].ap(), feats_d.ap())
                for c in range(3):
                    nc.sync.dma_start(dbg["emb_o"].ap()[c], embT[c][:])
                for dd2 in range(2):
                    nc.sync.dma_start(dbg["h_o"].ap()[dd2], hval[dd2][:])
                nc.sync.dma_start(dbg["ab_o"].ap(), ab_d.ap())

    nc.compile()
    return nc


# ---------------- host-side prep ----------------
def make_in_maps(inputs):
    inputs = {k: np.asarray(v) for k, v in inputs.items()}
    sentence = inputs["sentence"].astype(np.int64)
    chars = inputs["chars"].astype(np.int64)
    word_emb = np.concatenate(
        [inputs["word_emb"].astype(np.float32), np.zeros((1, E), np.float32)], 0)
    import ml_dtypes

    # shared param layouts
    CHUNKMAP = [0, 1, 2, 3, 6, 7, 4, 5]   # device gate chunks: i,f,o,g order

    def lhsT_tiles(w, kdim, kchunks, mchunks):
        # w: [out(gate), in] -> tiles [(kc, mc)] each [128(K), 128(M)]
        out = np.zeros((128, kchunks * mchunks * 128), np.float32)
        for kc in range(kchunks):
            for mc in range(mchunks):
                wb = CHUNKMAP[mc] if mchunks == 8 else mc
                blk = w[wb * 128:(wb + 1) * 128, kc * 128:(kc + 1) * 128]
                out[:, (kc * mchunks + mc) * 128:(kc * mchunks + mc) * 128 + 128] = blk.T
        return out

    w_ih_ext = {}
    for dd, (wn, bn) in enumerate((("w_ih_f", "b_f"), ("w_ih_b", "b_b"))):
        wext = np.zeros((4 * H, 384), np.float32)
        wext[:, :300] = inputs[wn][:, :300]
        wext[:, 320:345] = inputs[wn][:, 300:325]
        wext[:, 352] = inputs[bn]
        w_ih_ext[dd] = wext
    w_ihT = np.concatenate([lhsT_tiles(w_ih_ext[dd], 384, 3, 8) for dd in range(2)], 1)
    w_hhT = np.concatenate(
        [lhsT_tiles(inputs[wn], 256, 2, 8) for wn in ("w_hh_f", "w_hh_b")], 1
    ).astype(ml_dtypes.bfloat16)
    h2tT = np.zeros((128, 2 * 2 * 16), np.float32)
    for dd in range(2):
        for hc in range(2):
            blk = inputs["h2t_w"][:, dd * 256 + hc * 128: dd * 256 + (hc + 1) * 128]
            h2tT[:, (dd * 2 + hc) * 16:(dd * 2 + hc) * 16 + 12] = blk.T
    cembT = inputs["char_emb"].astype(np.float32).T.copy()           # [25, 85]
    convT = inputs["conv_w"][:, 0, :, :].transpose(2, 1, 0).reshape(CE, 3 * CO).copy()
    convb = inputs["conv_b"].astype(np.float32).reshape(CO, 1)
    trans = inputs["transitions"].astype(np.float32)
    trrep = np.tile(trans.reshape(1, K * K), (128, 1))
    ident = np.eye(128, dtype=np.float32)
    iota85 = np.arange(CV).reshape(CV, 1).astype(np.float32)

    in_maps = []
    for k in range(NC):
        m = {"word_emb": word_emb, "w_ihT": w_ihT, "w_hhT": w_hhT, "h2tT": h2tT,
             "cembT": cembT, "convT": convT, "convb": convb, "trrep": trrep,
             "ident": ident, "iota85": iota85}
        # gather indices + char grid
        widx = np.full((TEXT, 1), V, np.int32)
        charsb = np.full((TEXT, LP), PADCHAR, np.float32)
        for pr in range(TEXT):
            t = pr2t(k, pr)
            if 0 <= t < T:
                widx[pr, 0] = sentence[t]
                charsb[pr, 2:2 + L] = chars[t]
        m["widx"] = widx
        m["charsb"] = charsb.reshape(-1).astype(ml_dtypes.bfloat16)
        # beta transitions: transposed; core 7 masked for terminal STOP selection
        trb = trans.T.copy()
        if k == NC - 1:
            trb[START, STOP] += NEG
            trb[STOP, STOP] += NEG
        m["trrepb"] = np.tile(trb.reshape(1, K * K), (128, 1))
        # feats mask/patch
        fmask = np.ones((FD, K), np.float32)
        fpatch = np.zeros((FD, K), np.float32)
        if k == 0:
            fmask[FPAD + RV0 - 1] = 0.0
            fpatch[FPAD + RV0 - 1] = NEG
            fpatch[FPAD + RV0 - 1, START] = 0.0
        if k == NC - 1:
            r_T = (T - row2t(k, 0))  # feats row for t == T
            fmask[FPAD + r_T] = 0.0
            fpatch[FPAD + r_T] = NEG
            fpatch[FPAD + r_T, STOP] = 0.0
        m["fmask"] = fmask
        m["fpatch"] = fpatch
        m["trstop"] = (np.tile(trans[STOP].reshape(1, K), (128, 1))
                       if k == NC - 1 else np.zeros((128, K), np.float32))
        vm1 = np.zeros((128, 3), np.float32)
        vm2 = np.zeros((128, 3), np.float32)
        for g in range(3):
            for p in range(128):
                r = g * 128 + p
                if RV0 <= r < RV0 + B:
                    vm1[p, g] = 1.0
                    vm2[p, g] = 1.0
        m["vmask1"] = vm1
        m["vmask2"] = vm2
        vm3 = np.zeros((128, 1), np.float32)
        if k == NC - 1:
            vm3[RV0 - 1, 0] = 1.0
        m["vmask3"] = vm3
        in_maps.append(m)
    return in_maps


_NC_CACHE = {}


def kernel(**inputs):
    from concourse import bass_utils
    key = "main"
    if key not in _NC_CACHE:
        _NC_CACHE[key] = build_nc(debug_outputs=False)
    nc = _NC_CACHE[key]
    in_maps = make_in_maps(inputs)
    res = bass_utils.run_bass_kernel_spmd(nc, in_maps, core_ids=list(range(NC)))
    path = np.concatenate([res.results[k]["path_o"] for k in range(NC)]).astype(np.int32)
    score = np.float32(sum(np.float32(res.results[k]["score_o"].sum()) for k in range(NC)))
    return score, path


# revision 16
# speedup vs baseline: 1.1293x; 1.0033x over previous
"""BiLSTM-CRF Trainium2 kernel: 8-core SPMD, chunk-parallel LSTM + Viterbi.

Strategy (validated numerically against the reference in float32):
- Core k owns output slice [k*256, (k+1)*256) of the T=2048 sequence.
- The LSTM recurrence is chunk-parallelized: per core, per direction, 32
  streams each process a 14-step chunk preceded by a 40-step warmup from zero
  state (state influence decays ~4x/step, so the warmup converges to f32
  noise). Streams are batched in the matmul free dim, so one "superstep" does
  16 bf16 weight-tile matmuls for all 32 streams at once.
- Viterbi alpha (forward) and beta (backward) max-plus scans are
  chunk-parallelized the same way (128 streams x 3 steps, 56-step coalescence
  warmup); path[t] = argmax_i(alpha_t[i] + beta_t[i]).
- The score is recomputed from the decoded path (sum of edge scores) because
  chunked scans lose the global additive constant; per-core partials are
  summed on the host (part of unsharding).
- Sequence edges are handled with data only (all 8 cores run one program):
  virtual rows outside [0,T) use a zero word-embedding row and a pad char id;
  the alpha START init and the beta STOP init are injected via patched feats
  rows and a masked replicated transition matrix on the affected cores.

kernel(**inputs) -> (score, path) matching reference.reference().
"""
import numpy as np
from contextlib import ExitStack

import concourse.bass as bass
import concourse.tile as tile
from concourse import bacc, mybir

F32 = mybir.dt.float32
F32R = mybir.dt.float32r
BF16 = mybir.dt.bfloat16
I32 = mybir.dt.int32
AF = mybir.ActivationFunctionType
ALU = mybir.AluOpType
AX = mybir.AxisListType

# ---------------- geometry ----------------
T, K, H, E, CO, CE, CV, L = 2048, 12, 256, 300, 25, 25, 85, 20
V = 100000
START, STOP = 10, 11
NEG = -10000.0
NC = 8
B = T // NC          # 256

W = 28               # LSTM warmup steps
WV = 48              # Viterbi warmup steps
S = 64               # LSTM streams per direction
BF = 448             # feats rows per core; feats row r <-> t = k*B - 57 + r
B2 = BF // S         # 14
NSUP = B2 + W        # 54 LSTM supersteps
PR = BF + 2 * W      # 528 precomp rows; precomp row pr = feats row + W
TEXT = 640           # padded emb/precomp columns (5*128)
LP = 24              # padded word length
NCH = TEXT * LP
B2V = 3
NSUPV = WV + B2V     # 59
FPAD = 80            # junk rows below feats row 0 in DRAM buffers
FD = 608             # DRAM rows in feats/ab buffers (FPAD + 512 + 16)
PADCHAR = 200.0
RV0 = WV + 1         # feats row of t = k*B (first output row)
BROW = BF - NSUPV - 383         # beta view base row (NV = NSUPV)
BSROW = BF - 386 - WV           # beta stored base row


def row2t(k, r):     # feats row -> global t
    return k * B - (WV + 1) + r


def pr2t(k, pr):     # precomp/emb column -> global t
    return k * B - (WV + 1 + W) + pr


def _dap(dram, offset, pairs):
    """Raw strided AP over a DRAM tensor (element units)."""
    return bass.AP(dram, offset, [list(p) for p in pairs])


def build_nc(debug_outputs=False):
    nc = bacc.Bacc("TRN2", target_bir_lowering=False, debug=False, num_devices=NC)
    d = {}

    def ein(n, sh, dt):
        d[n] = nc.dram_tensor(n, sh, dt, kind="ExternalInput")

    ein("word_emb", [V + 1, E], F32)
    ein("widx", [TEXT, 1], I32)
    ein("charsb", [NCH], BF16)
    ein("iota85", [CV, 1], F32)
    ein("ident", [128, 128], F32)
    ein("w_ihT", [128, 2 * 3 * 8 * 128], F32)
    ein("w_hhT", [128, 2 * 2 * 8 * 128], BF16)
    ein("h2tT", [128, 2 * 2 * 16], F32)
    ein("cembT", [CE, CV], F32)
    ein("convT", [CE, 3 * CO], F32)
    ein("convb", [CO, 1], F32)
    ein("trrep", [128, K * K], F32)
    ein("trrepb", [128, K * K], F32)
    ein("fmask", [FD, K], F32)
    ein("fpatch", [FD, K], F32)
    ein("trstop", [128, K], F32)
    ein("vmask1", [128, 3], F32)
    ein("vmask2", [128, 3], F32)
    ein("vmask3", [128, 1], F32)

    path_o = nc.dram_tensor("path_o", [B], I32, kind="ExternalOutput")
    score_o = nc.dram_tensor("score_o", [128, 1], F32, kind="ExternalOutput")
    dbg = {}
    if debug_outputs:
        dbg["feats_o"] = nc.dram_tensor("feats_o", [FD, K], F32, kind="ExternalOutput")
        dbg["emb_o"] = nc.dram_tensor("emb_o", [3, 128, TEXT], F32, kind="ExternalOutput")
        dbg["h_o"] = nc.dram_tensor("h_o", [2, 128, 2 * BF], F32, kind="ExternalOutput")
        dbg["ab_o"] = nc.dram_tensor("ab_o", [2, FD, K], F32, kind="ExternalOutput")

    feats_d = nc.dram_tensor("feats_d", [FD, K], F32, kind="Internal")
    ab_d = nc.dram_tensor("ab_d", [2, FD, K], F32, kind="Internal")

    with tile.TileContext(nc) as tc, ExitStack() as ctx:
        sb = ctx.enter_context(tc.tile_pool(name="sb", bufs=1))

        # ---------- P0: params ----------
        w_ihT = sb.tile([128, 2 * 3 * 8 * 128], F32R)
        nc.gpsimd.dma_start(w_ihT[:], d["w_ihT"].ap())
        w_hhT = sb.tile([128, 2 * 2 * 8 * 128], BF16)
        nc.sync.dma_start(w_hhT[:], d["w_hhT"].ap())
        h2tT = sb.tile([128, 2 * 2 * 16], F32)
        nc.sync.dma_start(h2tT[:], d["h2tT"].ap())
        ident = sb.tile([128, 128], F32)
        nc.sync.dma_start(ident[:], d["ident"].ap())
        trrep = sb.tile([128, K * K], F32)
        nc.sync.dma_start(trrep[:], d["trrep"].ap())
        trrepb = sb.tile([128, K * K], F32)
        nc.sync.dma_start(trrepb[:], d["trrepb"].ap())
        trstop = sb.tile([128, K], F32)
        nc.sync.dma_start(trstop[:], d["trstop"].ap())
        vmask1 = sb.tile([128, 3], F32)
        nc.sync.dma_start(vmask1[:], d["vmask1"].ap())
        vmask2 = sb.tile([128, 3], F32)
        nc.sync.dma_start(vmask2[:], d["vmask2"].ap())
        vmask3 = sb.tile([128, 1], F32)
        nc.sync.dma_start(vmask3[:], d["vmask3"].ap())

        embT = [sb.tile([128, TEXT], F32, tag=f"embT{c}", name=f"embT{c}") for c in range(3)]
        embTr = [sb.tile([128, TEXT], F32R, tag=f"embTr{c}", name=f"embTr{c}") for c in range(3)]
        precompT = [sb.tile([128, 8 * TEXT], F32, tag=f"pre{dd}", name=f"pre{dd}") for dd in range(2)]
        hval = [sb.tile([128, 2 * BF], F32, tag=f"hval{dd}", name=f"hval{dd}") for dd in range(2)]

        # ---------- P1-P4: embeddings + charCNN ----------
        with tc.tile_pool(name="psA", bufs=2, space="PSUM") as psA, \
             tc.tile_pool(name="sbA", bufs=1) as sbA:
            iota85 = sbA.tile([CV, 1], F32, tag="iota85")
            nc.sync.dma_start(iota85[:], d["iota85"].ap())
            cembT = sbA.tile([CE, CV], F32, tag="cembT")
            nc.sync.dma_start(cembT[:], d["cembT"].ap())
            convT = sbA.tile([CE, 3 * CO], F32, tag="convT")
            nc.sync.dma_start(convT[:], d["convT"].ap())
            convb = sbA.tile([128, 1], F32, tag="convb")
            nc.sync.dma_start(convb[64:64 + CO, :], d["convb"].ap())

            ctab_ps = psA.tile([CV, 3 * CO], F32, space="PSUM", tag="ctab")
            nc.tensor.matmul(ctab_ps[:], cembT[:], convT[:], start=True, stop=True)
            ctab = sbA.tile([CV, 3 * CO], BF16, tag="ctabsb")
            nc.vector.tensor_copy(ctab[:], ctab_ps[:])

            onehot = sbA.tile([CV, NCH], BF16, tag="onehot")
            CHCH = 3840
            for c0 in range(0, NCH, CHCH):
                chbc = sbA.tile([CV, CHCH], BF16, tag="chb", bufs=2, name=f"chb{c0}")
                nc.sync.dma_start(chbc[:], _dap(d["charsb"], c0, [[0, CV], [1, CHCH]]))
                nc.vector.tensor_scalar(out=onehot[:, c0:c0 + CHCH], in0=chbc[:],
                                        scalar1=iota85[:], scalar2=None,
                                        op0=ALU.is_equal)

            for c in range(3):
                nc.vector.memset(embT[c][:], 0.0)

            widx = sbA.tile([128, 5], I32, tag="widx")
            nc.sync.dma_start(widx[:], _dap(d["widx"], 0, [[1, 128], [128, 5]]))
            for g in range(5):
                wrows = sbA.tile([128, 304], F32, tag="wrows", bufs=2, name=f"wrows{g}")
                nc.gpsimd.indirect_dma_start(
                    out=wrows[:, :300], out_offset=None, in_=d["word_emb"].ap(),
                    in_offset=bass.IndirectOffsetOnAxis(ap=widx[:, g:g + 1], axis=0))
                for c in range(3):
                    dim0 = c * 128
                    ncols = min(128, 300 - dim0)
                    tp = psA.tile([128, 128], F32, space="PSUM", tag="tp")
                    nc.tensor.transpose(tp[:ncols, :], wrows[:, dim0:dim0 + ncols],
                                        ident[:])
                    nc.vector.tensor_copy(embT[c][:ncols, g * 128:(g + 1) * 128],
                                          tp[:ncols, :128])
            nc.vector.memset(embT[2][96:97, :], 1.0)

            # charCNN: co[d,(t,j)] = sum_k Tk-matmul over windowed onehot
            TCW = 23
            oh3 = onehot[:].rearrange("c (t l) -> c t l", l=LP)
            nsl = (TEXT + TCW - 1) // TCW
            for sl in range(nsl):
                t0 = sl * TCW
                tn = min(TCW, TEXT - t0)
                co_ps = psA.tile([128, TCW * (L + 2)], F32, space="PSUM", tag="co")
                co3 = co_ps[64:64 + CO, :tn * (L + 2)].rearrange("d (t j) -> d t j", j=L + 2)
                for k3 in range(3):
                    nc.tensor.matmul(co3, ctab[:, k3 * CO:(k3 + 1) * CO],
                                     oh3[:, t0:t0 + tn, k3:k3 + L + 2],
                                     start=(k3 == 0), stop=(k3 == 2))
                cf = sbA.tile([128, TCW], F32, tag="cf", bufs=2, name=f"cf{sl}")
                nc.vector.tensor_reduce(cf[64:64 + CO, :tn], co3, axis=AX.X, op=ALU.max)
                nc.vector.tensor_scalar(out=embT[2][64:64 + CO, t0:t0 + tn],
                                        in0=cf[64:64 + CO, :tn],
                                        scalar1=convb[64:64 + CO, :],
                                        scalar2=None, op0=ALU.add)

            for c in range(3):
                nc.vector.tensor_copy(embTr[c][:], embT[c][:])

        # ---------- P5: input matmuls ----------
        with tc.tile_pool(name="psB", bufs=2, space="PSUM") as psB:
            for dd in range(2):
                for mc in range(8):
                    pc_ps = psB.tile([128, TEXT], F32, space="PSUM", tag="pcps")
                    for n0, nn in ((0, 512), (512, 128)):
                        for kc in range(3):
                            w0 = ((dd * 3 + kc) * 8 + mc) * 128
                            nc.tensor.matmul(pc_ps[:, n0:n0 + nn],
                                             w_ihT[:, w0:w0 + 128],
                                             embTr[kc][:, n0:n0 + nn],
                                             start=(kc == 0), stop=(kc == 2))
                    nc.scalar.activation(precompT[dd][:, mc * TEXT:(mc + 1) * TEXT],
                                         pc_ps[:], AF.Copy)

        # ---------- P6: LSTM supersteps ----------
        def pr_ap(dd, u):
            t3 = precompT[dd][:].rearrange("p (c r) -> p c r", c=8)
            if dd == 0:
                return t3[:, :, u:u + (S - 1) * B2 + 1:B2]
            base = PR - 1 - u
            return t3[:, :, base - (S - 1) * B2:base + 1:B2]

        def hv_ap(dd, u):
            t3 = hval[dd][:].rearrange("p (c r) -> p c r", c=2)
            if dd == 0:
                r0 = u - W
                return t3[:, :, r0:r0 + (S - 1) * B2 + 1:B2]
            base = BF - 1 - (u - W)
            return t3[:, :, base - (S - 1) * B2:base + 1:B2]

        with tc.tile_pool(name="psC", bufs=2, space="PSUM") as psC, \
             tc.tile_pool(name="sbC", bufs=1) as sbC:
            hscr = [sbC.tile([128, 2 * S], F32, tag=f"hscr{dd}", name=f"hscr{dd}") for dd in range(2)]
            hbf = [sbC.tile([128, 2 * S], BF16, tag=f"hbf{dd}", name=f"hbf{dd}") for dd in range(2)]
            cstA = [sbC.tile([128, 2 * S], F32, tag=f"cstA{dd}", name=f"cstA{dd}") for dd in range(2)]
            cstB = [sbC.tile([128, 2 * S], F32, tag=f"cstB{dd}", name=f"cstB{dd}") for dd in range(2)]
            gsb = [sbC.tile([128, 8 * S], F32, tag=f"gsb{dd}", name=f"gsb{dd}") for dd in range(2)]
            sgo = [sbC.tile([128, 6 * S], F32, tag=f"sgo{dd}", name=f"sgo{dd}") for dd in range(2)]
            tgg = [sbC.tile([128, 2 * S], F32, tag=f"tgg{dd}", name=f"tgg{dd}") for dd in range(2)]
            tg2 = [sbC.tile([128, 2 * S], F32, tag=f"tg2{dd}", name=f"tg2{dd}") for dd in range(2)]
            tm1 = [sbC.tile([128, 2 * S], F32, tag=f"tm1{dd}", name=f"tm1{dd}") for dd in range(2)]
            for dd in range(2):
                nc.vector.memset(hbf[dd][:], 0.0)
                nc.vector.memset(cstA[dd][:], 0.0)
                nc.vector.memset(hval[dd][:], 0.0)

            for u in range(NSUP):
                for dd in range(2):
                    gates = psC.tile([128, 8 * S], F32, space="PSUM", tag=f"g{dd}")
                    for mc in range(8):
                        for kc in range(2):
                            w0 = ((dd * 2 + kc) * 8 + mc) * 128
                            nc.tensor.matmul(gates[:, mc * S:(mc + 1) * S],
                                             w_hhT[:, w0:w0 + 128],
                                             hbf[dd][:, kc * S:(kc + 1) * S],
                                             start=(kc == 0), stop=(kc == 1))
                    g = gsb[dd]
                    nc.vector.tensor_tensor(
                        out=g[:].rearrange("p (c r) -> p c r", c=8),
                        in0=gates[:].rearrange("p (c r) -> p c r", c=8),
                        in1=pr_ap(dd, u), op=ALU.add)
                    nc.scalar.activation(sgo[dd][:], g[:, :6 * S], AF.Sigmoid)
                    nc.scalar.activation(tgg[dd][:], g[:, 6 * S:8 * S], AF.Sigmoid,
                                         scale=2.0)
                    nc.vector.tensor_scalar(out=tg2[dd][:], in0=tgg[dd][:],
                                            scalar1=2.0, scalar2=-1.0,
                                            op0=ALU.mult, op1=ALU.add)
                    nc.vector.tensor_mul(tm1[dd][:], sgo[dd][:, :2 * S], tg2[dd][:])
                    nc.vector.tensor_mul(cstB[dd][:], sgo[dd][:, 2 * S:4 * S],
                                         cstA[dd][:])
                    nc.vector.tensor_add(cstA[dd][:], cstB[dd][:], tm1[dd][:])
                    nc.scalar.activation(tgg[dd][:], cstA[dd][:], AF.Sigmoid, scale=2.0)
                    nc.vector.tensor_scalar(out=tg2[dd][:], in0=tgg[dd][:],
                                            scalar1=2.0, scalar2=-1.0,
                                            op0=ALU.mult, op1=ALU.add)
                    hdst = (hv_ap(dd, u) if u >= W
                            else hscr[dd][:].rearrange("p (c r) -> p c r", c=2))
                    nc.vector.tensor_tensor(
                        out=hdst,
                        in0=sgo[dd][:, 4 * S:6 * S].rearrange("p (c r) -> p c r", c=2),
                        in1=tg2[dd][:].rearrange("p (c r) -> p c r", c=2), op=ALU.mult)
                    nc.vector.tensor_copy(
                        hbf[dd][:].rearrange("p (c r) -> p c r", c=2), hdst)

        # ---------- P7: feats ----------
        with tc.tile_pool(name="psD", bufs=2, space="PSUM") as psD, \
             tc.tile_pool(name="sbD", bufs=2) as sbD:
            fsb = sb.tile([16, FPAD + 512], F32)
            nc.vector.memset(fsb[:], 0.0)
            f_ps = psD.tile([16, 448], F32, space="PSUM", tag="fps")
            first = True
            for dd in range(2):
                for hc in range(2):
                    wt = h2tT[:, (dd * 2 + hc) * 16:(dd * 2 + hc) * 16 + 16]
                    nc.tensor.matmul(f_ps[:, :BF], wt, hval[dd][:, hc * BF:(hc + 1) * BF],
                                     start=first, stop=(dd == 1 and hc == 1))
                    first = False
            nc.vector.tensor_copy(fsb[:, FPAD:FPAD + BF], f_ps[:, :BF])
            fz = sb.tile([128, K], F32)
            nc.vector.memset(fz[:], 0.0)
            nc.sync.dma_start(_dap(feats_d, 0, [[K, FPAD], [1, K]]), fz[:FPAD, :])
            nc.sync.dma_start(_dap(feats_d, (FPAD + BF) * K, [[K, FD - FPAD - BF], [1, K]]),
                              fz[:FD - FPAD - BF, :])
            for g in range(4):
                tp2 = psD.tile([128, 16], F32, space="PSUM", tag="ftp")
                nc.tensor.transpose(tp2[:, :16], fsb[:16, FPAD + g * 128:FPAD + (g + 1) * 128],
                                    ident[:16, :16])
                ftile = sbD.tile([128, K], F32, tag="ftile")
                nc.vector.tensor_copy(ftile[:], tp2[:, :K])
                nc.sync.dma_start(
                    _dap(feats_d, (FPAD + g * 128) * K, [[K, 128], [1, K]]), ftile[:])

        # ---------- P8: viterbi feats views + patch ----------
        NV = NSUPV
        with tc.tile_pool(name="sbE", bufs=1) as sbE, \
             tc.tile_pool(name="psE", bufs=2, space="PSUM") as psE:
            fva = sbE.tile([128, NV * K], F32, tag="fva")
            fvb = sbE.tile([128, NV * K], F32, tag="fvb")
            vscr = sbE.tile([128, NV * K], F32, tag="vscr")
            vmsk = sbE.tile([128, NV * K], F32, tag="vmsk")
            vpat = sbE.tile([128, NV * K], F32, tag="vpat")
            AOFF, ASTEP, AUST = FPAD * K, B2V * K, K
            BOFF, BSTEP, BUST = (FPAD + BROW) * K, B2V * K, K
            for vt, off, pstep, ustep in ((fva, AOFF, ASTEP, AUST),
                                          (fvb, BOFF, BSTEP, BUST)):
                nc.sync.dma_start(vscr[:].rearrange("p (u k) -> p u k", k=K),
                                  _dap(feats_d, off, [[pstep, 128], [ustep, NV], [1, K]]))
                nc.sync.dma_start(vmsk[:].rearrange("p (u k) -> p u k", k=K),
                                  _dap(d["fmask"], off, [[pstep, 128], [ustep, NV], [1, K]]))
                nc.sync.dma_start(vpat[:].rearrange("p (u k) -> p u k", k=K),
                                  _dap(d["fpatch"], off, [[pstep, 128], [ustep, NV], [1, K]]))
                nc.vector.tensor_mul(vmsk[:], vscr[:], vmsk[:])
                nc.vector.tensor_add(vt[:], vmsk[:], vpat[:])

            # ---------- P9: alpha/beta scans ----------
            absb = sbE.tile([128, 2 * 3 * K], F32, tag="absb")
            for which, fview, trr in ((0, fva, trrep), (1, fvb, trrepb)):
                fv = sbE.tile([128, K], F32, tag=f"fv{which}", name=f"fv{which}")
                fvR = sbE.tile([128, K], F32, tag=f"fvR{which}", name=f"fvR{which}")
                nc.vector.memset(fv[:], 0.0)
                tmpv = sbE.tile([128, K * K], F32, tag=f"tmpv{which}", name=f"tmpv{which}")
                f3 = fview[:].rearrange("p (u k) -> p u k", k=K)
                eng = nc.vector if which == 0 else nc.gpsimd
                for u in range(NV):
                    uf = u if which == 0 else (NV - 1 - u)
                    eng.tensor_tensor(
                        out=tmpv[:].rearrange("p (i j) -> p i j", i=K),
                        in0=fv[:].rearrange("p (o j) -> p o j", o=1).to_broadcast([128, K, K]),
                        in1=trr[:].rearrange("p (i j) -> p i j", i=K), op=ALU.add)
                    nc.vector.tensor_reduce(fvR[:], tmpv[:].rearrange(
                        "p (i j) -> p i j", i=K), axis=AX.X, op=ALU.max)
                    eng.tensor_tensor(out=fv[:], in0=fvR[:], in1=f3[:, uf, :],
                                      op=ALU.add)
                    if WV <= u < WV + 3:
                        slot = which * 3 + ((u - WV) if which == 0 else (WV + 2 - u))
                        nc.vector.tensor_copy(absb[:, slot * K:(slot + 1) * K], fv[:])
            zt = sbE.tile([128, K], F32, tag="zt")
            nc.vector.memset(zt[:], 0.0)
            for which, zhi in ((0, FPAD + WV), (1, FPAD + BSROW)):
                base = which * FD * K
                for z0 in range(0, zhi, 128):
                    zn = min(128, zhi - z0)
                    nc.sync.dma_start(_dap(ab_d, base + z0 * K, [[K, zn], [1, K]]),
                                      zt[:zn, :])
                nc.sync.dma_start(
                    _dap(ab_d, base + (FPAD + BF) * K, [[K, FD - FPAD - BF], [1, K]]),
                    zt[:FD - FPAD - BF, :])
            # alpha rows: n*3 + 56 + s ; beta rows: 390 - 3n - s
            nc.sync.dma_start(
                _dap(ab_d, (FPAD + WV) * K, [[3 * K, 128], [K, 3], [1, K]]),
                absb[:, 0:3 * K].rearrange("p (s k) -> p s k", k=K))
            nc.sync.dma_start(
                _dap(ab_d, FD * K + (FPAD + BSROW) * K, [[3 * K, 128], [K, 3], [1, K]]),
                absb[:, 3 * K:6 * K].rearrange("p (s k) -> p s k", k=K))

            # ---------- P10: path + score ----------
            def aligned(src, base, shift):
                return _dap(src, base + (FPAD + shift) * K, [[K, 128], [128 * K, 3], [1, K]])

            al_al = sbE.tile([128, 3 * K], F32, tag="alal")
            be_al = sbE.tile([128, 3 * K], F32, tag="beal")
            al_p = sbE.tile([128, 3 * K], F32, tag="alp")
            be_p = sbE.tile([128, 3 * K], F32, tag="bep")
            ft_al = sbE.tile([128, 3 * K], F32, tag="ftal")
            ft_p = sbE.tile([128, 3 * K], F32, tag="ftp2")
            for dst, src, base, sh in ((al_al, ab_d, 0, 0), (be_al, ab_d, FD * K, 0),
                                       (al_p, ab_d, 0, -1), (be_p, ab_d, FD * K, -1),
                                       (ft_al, feats_d, 0, 0), (ft_p, feats_d, 0, -1)):
                nc.sync.dma_start(dst[:].rearrange("p (g k) -> p g k", k=K),
                                  aligned(src, base, sh))
            tot = sbE.tile([128, 3 * K], F32, tag="tot")
            totp = sbE.tile([128, 3 * K], F32, tag="totp")
            gsc = sbE.tile([128, 3 * K], F32, tag="gsc")
            # tot = alpha + (gamma - feats)   (beta scan state is gamma = beta + feats)
            nc.vector.tensor_tensor(out=gsc[:], in0=be_al[:], in1=ft_al[:], op=ALU.subtract)
            nc.vector.tensor_add(tot[:], al_al[:], gsc[:])
            nc.vector.tensor_tensor(out=gsc[:], in0=be_p[:], in1=ft_p[:], op=ALU.subtract)
            nc.vector.tensor_add(totp[:], al_p[:], gsc[:])
            mx = sbE.tile([128, 3], F32, tag="mx")
            mxp = sbE.tile([128, 3], F32, tag="mxp")
            msk = sbE.tile([128, 3 * K], F32, tag="msk")
            mskp = sbE.tile([128, 3 * K], F32, tag="mskp")
            for mm, tt2, mk in ((mx, tot, msk), (mxp, totp, mskp)):
                nc.vector.tensor_reduce(mm[:], tt2[:].rearrange("p (g k) -> p g k", k=K),
                                        axis=AX.X, op=ALU.max)
                nc.vector.tensor_tensor(
                    out=mk[:].rearrange("p (g k) -> p g k", k=K),
                    in0=tt2[:].rearrange("p (g k) -> p g k", k=K),
                    in1=mm[:].rearrange("p (g o) -> p g o", o=1).to_broadcast([128, 3, K]),
                    op=ALU.is_ge)
            iotaK = sbE.tile([128, K], F32, tag="iotaK")
            nc.gpsimd.iota(iotaK[:], pattern=[[1, K]], base=0, channel_multiplier=0,
                           allow_small_or_imprecise_dtypes=True)
            wrk = sbE.tile([128, 3 * K], F32, tag="wrk")
            nc.vector.tensor_scalar(out=wrk[:], in0=iotaK[:].rearrange(
                "p (o k) -> p o k", o=1).to_broadcast([128, 3, K]), scalar1=-1.0,
                scalar2=float(K - 1), op0=ALU.mult, op1=ALU.add)
            wrk2 = sbE.tile([128, 3 * K], F32, tag="wrk2")
            nc.vector.tensor_mul(wrk2[:], wrk[:], msk[:])
            pathf = sbE.tile([128, 3], F32, tag="pathf")
            pathf2 = sbE.tile([128, 3], F32, tag="pathf2")
            nc.vector.tensor_reduce(pathf2[:], wrk2[:].rearrange("p (g k) -> p g k", k=K),
                                    axis=AX.X, op=ALU.max)
            nc.vector.tensor_scalar(out=pathf[:], in0=pathf2[:], scalar1=-1.0,
                                    scalar2=float(K - 1), op0=ALU.mult, op1=ALU.add)
            pathi = sbE.tile([128, 3], I32, tag="pathi")
            nc.vector.tensor_copy(pathi[:], pathf[:])
            # rows r in [57, 313) -> path_o[0:256]
            n0 = 128 - RV0
            nc.sync.dma_start(_dap(path_o, 0, [[1, n0]]), pathi[RV0:128, 0:1])
            nc.sync.dma_start(_dap(path_o, n0, [[1, 128]]), pathi[:, 1:2])
            nc.sync.dma_start(_dap(path_o, n0 + 128, [[1, RV0]]), pathi[0:RV0, 2:3])

            # score partials
            sc0 = sbE.tile([128, 3 * K], F32, tag="sc0")
            sc1 = sbE.tile([128, 3 * K], F32, tag="sc1")
            nc.vector.tensor_mul(sc0[:], ft_al[:], msk[:])
            nc.vector.tensor_tensor(
                out=sc1[:].rearrange("p (g k) -> p g k", k=K),
                in0=sc0[:].rearrange("p (g k) -> p g k", k=K),
                in1=vmask1[:].rearrange("p (g o) -> p g o", o=1).to_broadcast([128, 3, K]),
                op=ALU.mult)
            w12 = sbE.tile([128, 3 * K * K], F32, tag="w12")
            nc.vector.tensor_tensor(
                out=w12[:].rearrange("p (g i j) -> p g i j", i=K, j=K),
                in0=trrep[:].rearrange("p (o i j) -> p o i j", o=1, i=K).to_broadcast([128, 3, K, K]),
                in1=mskp[:].rearrange("p (g o j) -> p g o j", o=1, j=K).to_broadcast([128, 3, K, K]), op=ALU.mult)
            wred0 = sbE.tile([128, 3 * K], F32, tag="wred0")
            wred1 = sbE.tile([128, 3 * K], F32, tag="wred1")
            wred = sbE.tile([128, 3 * K], F32, tag="wred")
            nc.vector.tensor_reduce(wred0[:], w12[:].rearrange(
                "p (g i j) -> p g i j", i=K, j=K), axis=AX.X, op=ALU.add)
            nc.vector.tensor_mul(wred1[:], wred0[:], msk[:])
            nc.vector.tensor_tensor(
                out=wred[:].rearrange("p (g k) -> p g k", k=K),
                in0=wred1[:].rearrange("p (g k) -> p g k", k=K),
                in1=vmask2[:].rearrange("p (g o) -> p g o", o=1).to_broadcast([128, 3, K]),
                op=ALU.mult)
            s3 = sbE.tile([128, K], F32, tag="s3")
            nc.vector.tensor_mul(s3[:], trstop[:], msk[:, 2 * K:3 * K])
            acc = sbE.tile([128, 3 * K], F32, tag="acc")
            nc.vector.tensor_add(acc[:], sc1[:], wred[:])
            stot = sbE.tile([128, 1], F32, tag="stot")
            nc.vector.tensor_reduce(stot[:], acc[:], axis=AX.X, op=ALU.add)
            s3r = sbE.tile([128, 1], F32, tag="s3r")
            s3m = sbE.tile([128, 1], F32, tag="s3m")
            stot2 = sbE.tile([128, 1], F32, tag="stot2")
            nc.vector.tensor_reduce(s3r[:], s3[:], axis=AX.X, op=ALU.add)
            nc.vector.tensor_mul(s3m[:], s3r[:], vmask3[:])
            nc.vector.tensor_add(stot2[:], stot[:], s3m[:])
            nc.sync.dma_start(score_o.ap(), stot2[:])

            if debug_outputs:
                nc.sync.dma_start(dbg["feats_o"].ap(), feats_d.ap())
                for c in range(3):
                    nc.sync.dma_start(dbg["emb_o"].ap()[c], embT[c][:])
                for dd2 in range(2):
                    nc.sync.dma_start(dbg["h_o"].ap()[dd2], hval[dd2][:])
                nc.sync.dma_start(dbg["ab_o"].ap(), ab_d.ap())

    nc.compile()
    return nc


# ---------------- host-side prep ----------------
def make_in_maps(inputs):
    inputs = {k: np.asarray(v) for k, v in inputs.items()}
    sentence = inputs["sentence"].astype(np.int64)
    chars = inputs["chars"].astype(np.int64)
    word_emb = np.concatenate(
        [inputs["word_emb"].astype(np.float32), np.zeros((1, E), np.float32)], 0)
    import ml_dtypes

    # shared param layouts
    CHUNKMAP = [0, 1, 2, 3, 6, 7, 4, 5]   # device gate chunks: i,f,o,g order

    def lhsT_tiles(w, kdim, kchunks, mchunks):
        # w: [out(gate), in] -> tiles [(kc, mc)] each [128(K), 128(M)]
        out = np.zeros((128, kchunks * mchunks * 128), np.float32)
        for kc in range(kchunks):
            for mc in range(mchunks):
                wb = CHUNKMAP[mc] if mchunks == 8 else mc
                blk = w[wb * 128:(wb + 1) * 128, kc * 128:(kc + 1) * 128]
                out[:, (kc * mchunks + mc) * 128:(kc * mchunks + mc) * 128 + 128] = blk.T
        return out

    w_ih_ext = {}
    for dd, (wn, bn) in enumerate((("w_ih_f", "b_f"), ("w_ih_b", "b_b"))):
        wext = np.zeros((4 * H, 384), np.float32)
        wext[:, :300] = inputs[wn][:, :300]
        wext[:, 320:345] = inputs[wn][:, 300:325]
        wext[:, 352] = inputs[bn]
        w_ih_ext[dd] = wext
    w_ihT = np.concatenate([lhsT_tiles(w_ih_ext[dd], 384, 3, 8) for dd in range(2)], 1)
    w_hhT = np.concatenate(
        [lhsT_tiles(inputs[wn], 256, 2, 8) for wn in ("w_hh_f", "w_hh_b")], 1
    ).astype(ml_dtypes.bfloat16)
    h2tT = np.zeros((128, 2 * 2 * 16), np.float32)
    for dd in range(2):
        for hc in range(2):
            blk = inputs["h2t_w"][:, dd * 256 + hc * 128: dd * 256 + (hc + 1) * 128]
            h2tT[:, (dd * 2 + hc) * 16:(dd * 2 + hc) * 16 + 12] = blk.T
    cembT = inputs["char_emb"].astype(np.float32).T.copy()           # [25, 85]
    convT = inputs["conv_w"][:, 0, :, :].transpose(2, 1, 0).reshape(CE, 3 * CO).copy()
    convb = inputs["conv_b"].astype(np.float32).reshape(CO, 1)
    trans = inputs["transitions"].astype(np.float32)
    trrep = np.tile(trans.reshape(1, K * K), (128, 1))
    ident = np.eye(128, dtype=np.float32)
    iota85 = np.arange(CV).reshape(CV, 1).astype(np.float32)

    in_maps = []
    for k in range(NC):
        m = {"word_emb": word_emb, "w_ihT": w_ihT, "w_hhT": w_hhT, "h2tT": h2tT,
             "cembT": cembT, "convT": convT, "convb": convb, "trrep": trrep,
             "ident": ident, "iota85": iota85}
        # gather indices + char grid
        widx = np.full((TEXT, 1), V, np.int32)
        charsb = np.full((TEXT, LP), PADCHAR, np.float32)
        for pr in range(TEXT):
            t = pr2t(k, pr)
            if 0 <= t < T:
                widx[pr, 0] = sentence[t]
                charsb[pr, 2:2 + L] = chars[t]
        m["widx"] = widx
        m["charsb"] = charsb.reshape(-1).astype(ml_dtypes.bfloat16)
        # beta transitions: transposed; core 7 masked for terminal STOP selection
        trb = trans.T.copy()
        if k == NC - 1:
            trb[START, STOP] += NEG
            trb[STOP, STOP] += NEG
        m["trrepb"] = np.tile(trb.reshape(1, K * K), (128, 1))
        # feats mask/patch
        fmask = np.ones((FD, K), np.float32)
        fpatch = np.zeros((FD, K), np.float32)
        if k == 0:
            fmask[FPAD + RV0 - 1] = 0.0
            fpatch[FPAD + RV0 - 1] = NEG
            fpatch[FPAD + RV0 - 1, START] = 0.0
        if k == NC - 1:
            r_T = (T - row2t(k, 0))  # feats row for t == T
            fmask[FPAD + r_T] = 0.0
            fpatch[FPAD + r_T] = NEG
            fpatch[FPAD + r_T, STOP] = 0.0
        m["fmask"] = fmask
        m["fpatch"] = fpatch
        m["trstop"] = (np.tile(trans[STOP].reshape(1, K), (128, 1))
                       if k == NC - 1 else np.zeros((128, K), np.float32))
        vm1 = np.zeros((128, 3), np.float32)
        vm2 = np.zeros((128, 3), np.float32)
        for g in range(3):
            for p in range(128):
                r = g * 128 + p
                if RV0 <= r < RV0 + B:
                    vm1[p, g] = 1.0
                    vm2[p, g] = 1.0
        m["vmask1"] = vm1
        m["vmask2"] = vm2
        vm3 = np.zeros((128, 1), np.float32)
        if k == NC - 1:
            vm3[RV0 - 1, 0] = 1.0
        m["vmask3"] = vm3
        in_maps.append(m)
    return in_maps


_NC_CACHE = {}


def kernel(**inputs):
    from concourse import bass_utils
    key = "main"
    if key not in _NC_CACHE:
        _NC_CACHE[key] = build_nc(debug_outputs=False)
    nc = _NC_CACHE[key]
    in_maps = make_in_maps(inputs)
    res = bass_utils.run_bass_kernel_spmd(nc, in_maps, core_ids=list(range(NC)))
    path = np.concatenate([res.results[k]["path_o"] for k in range(NC)]).astype(np.int32)
    score = np.float32(sum(np.float32(res.results[k]["score_o"].sum()) for k in range(NC)))
    return score, path


# revision 17
# speedup vs baseline: 1.1894x; 1.0532x over previous
"""BiLSTM-CRF Trainium2 kernel: 8-core SPMD, chunk-parallel LSTM + Viterbi.

Strategy (validated numerically against the reference in float32):
- Core k owns output slice [k*256, (k+1)*256) of the T=2048 sequence.
- The LSTM recurrence is chunk-parallelized: per core, per direction, 32
  streams each process a 14-step chunk preceded by a 40-step warmup from zero
  state (state influence decays ~4x/step, so the warmup converges to f32
  noise). Streams are batched in the matmul free dim, so one "superstep" does
  16 bf16 weight-tile matmuls for all 32 streams at once.
- Viterbi alpha (forward) and beta (backward) max-plus scans are
  chunk-parallelized the same way (128 streams x 3 steps, 56-step coalescence
  warmup); path[t] = argmax_i(alpha_t[i] + beta_t[i]).
- The score is recomputed from the decoded path (sum of edge scores) because
  chunked scans lose the global additive constant; per-core partials are
  summed on the host (part of unsharding).
- Sequence edges are handled with data only (all 8 cores run one program):
  virtual rows outside [0,T) use a zero word-embedding row and a pad char id;
  the alpha START init and the beta STOP init are injected via patched feats
  rows and a masked replicated transition matrix on the affected cores.

kernel(**inputs) -> (score, path) matching reference.reference().
"""
import numpy as np
from contextlib import ExitStack

import concourse.bass as bass
import concourse.tile as tile
from concourse import bacc, mybir

F32 = mybir.dt.float32
F32R = mybir.dt.float32r
BF16 = mybir.dt.bfloat16
I32 = mybir.dt.int32
AF = mybir.ActivationFunctionType
ALU = mybir.AluOpType
AX = mybir.AxisListType

# ---------------- geometry ----------------
T, K, H, E, CO, CE, CV, L = 2048, 12, 256, 300, 25, 25, 85, 20
V = 100000
START, STOP = 10, 11
NEG = -10000.0
NC = 8
B = T // NC          # 256

W = 28               # LSTM warmup steps
WV = 48              # Viterbi warmup steps
S = 64               # LSTM streams per direction
BF = 448             # feats rows per core; feats row r <-> t = k*B - 57 + r
B2 = BF // S         # 14
NSUP = B2 + W        # 54 LSTM supersteps
PR = BF + 2 * W      # 528 precomp rows; precomp row pr = feats row + W
TEXT = 640           # padded emb/precomp columns (5*128)
LP = 24              # padded word length
NCH = TEXT * LP
B2V = 3
NSUPV = WV + B2V     # 59
FPAD = 80            # junk rows below feats row 0 in DRAM buffers
FD = 608             # DRAM rows in feats/ab buffers (FPAD + 512 + 16)
PADCHAR = 200.0
RV0 = WV + 1         # feats row of t = k*B (first output row)
BROW = BF - NSUPV - 383         # beta view base row (NV = NSUPV)
BSROW = BF - 386 - WV           # beta stored base row


def row2t(k, r):     # feats row -> global t
    return k * B - (WV + 1) + r


def pr2t(k, pr):     # precomp/emb column -> global t
    return k * B - (WV + 1 + W) + pr


def _dap(dram, offset, pairs):
    """Raw strided AP over a DRAM tensor (element units)."""
    return bass.AP(dram, offset, [list(p) for p in pairs])


def build_nc(debug_outputs=False):
    nc = bacc.Bacc("TRN2", target_bir_lowering=False, debug=False, num_devices=NC)
    d = {}

    def ein(n, sh, dt):
        d[n] = nc.dram_tensor(n, sh, dt, kind="ExternalInput")

    ein("word_emb", [V + 1, E], F32)
    ein("widx", [TEXT, 1], I32)
    ein("charsb", [NCH], BF16)
    ein("iota85", [CV, 1], F32)
    ein("ident", [128, 128], F32)
    ein("w_ihT", [128, 2 * 3 * 8 * 128], F32)
    ein("w_hhT", [128, 2 * 2 * 8 * 128], BF16)
    ein("h2tT", [128, 2 * 2 * 16], F32)
    ein("cembT", [CE, CV], F32)
    ein("convT", [CE, 3 * CO], F32)
    ein("convb", [CO, 1], F32)
    ein("trrep", [128, K * K], F32)
    ein("trrepb", [128, K * K], F32)
    ein("fmask", [FD, K], F32)
    ein("fpatch", [FD, K], F32)
    ein("trstop", [128, K], F32)
    ein("vmask1", [128, 3], F32)
    ein("vmask2", [128, 3], F32)
    ein("vmask3", [128, 1], F32)

    path_o = nc.dram_tensor("path_o", [B], I32, kind="ExternalOutput")
    score_o = nc.dram_tensor("score_o", [128, 1], F32, kind="ExternalOutput")
    dbg = {}
    if debug_outputs:
        dbg["feats_o"] = nc.dram_tensor("feats_o", [FD, K], F32, kind="ExternalOutput")
        dbg["emb_o"] = nc.dram_tensor("emb_o", [3, 128, TEXT], F32, kind="ExternalOutput")
        dbg["h_o"] = nc.dram_tensor("h_o", [2, 128, 2 * BF], F32, kind="ExternalOutput")
        dbg["ab_o"] = nc.dram_tensor("ab_o", [2, FD, K], F32, kind="ExternalOutput")

    feats_d = nc.dram_tensor("feats_d", [FD, K], F32, kind="Internal")
    ab_d = nc.dram_tensor("ab_d", [2, FD, K], F32, kind="Internal")

    with tile.TileContext(nc) as tc, ExitStack() as ctx:
        sb = ctx.enter_context(tc.tile_pool(name="sb", bufs=1))

        # ---------- P0: params ----------
        w_ihT = sb.tile([128, 2 * 3 * 8 * 128], F32R)
        nc.gpsimd.dma_start(w_ihT[:], d["w_ihT"].ap())
        w_hhT = sb.tile([128, 2 * 2 * 8 * 128], BF16)
        nc.sync.dma_start(w_hhT[:], d["w_hhT"].ap())
        h2tT = sb.tile([128, 2 * 2 * 16], F32)
        nc.sync.dma_start(h2tT[:], d["h2tT"].ap())
        ident = sb.tile([128, 128], F32)
        nc.sync.dma_start(ident[:], d["ident"].ap())
        trrep = sb.tile([128, K * K], F32)
        nc.sync.dma_start(trrep[:], d["trrep"].ap())
        trrepb = sb.tile([128, K * K], F32)
        nc.sync.dma_start(trrepb[:], d["trrepb"].ap())
        trstop = sb.tile([128, K], F32)
        nc.sync.dma_start(trstop[:], d["trstop"].ap())
        vmask1 = sb.tile([128, 3], F32)
        nc.sync.dma_start(vmask1[:], d["vmask1"].ap())
        vmask2 = sb.tile([128, 3], F32)
        nc.sync.dma_start(vmask2[:], d["vmask2"].ap())
        vmask3 = sb.tile([128, 1], F32)
        nc.sync.dma_start(vmask3[:], d["vmask3"].ap())

        embT = [sb.tile([128, TEXT], F32, tag=f"embT{c}", name=f"embT{c}") for c in range(3)]
        embTr = [sb.tile([128, TEXT], F32R, tag=f"embTr{c}", name=f"embTr{c}") for c in range(3)]
        precompT = [sb.tile([128, 8 * TEXT], F32, tag=f"pre{dd}", name=f"pre{dd}") for dd in range(2)]
        hval = [sb.tile([128, 2 * BF], F32, tag=f"hval{dd}", name=f"hval{dd}") for dd in range(2)]

        # ---------- P1-P4: embeddings + charCNN ----------
        with tc.tile_pool(name="psA", bufs=2, space="PSUM") as psA, \
             tc.tile_pool(name="sbA", bufs=1) as sbA:
            iota85 = sbA.tile([CV, 1], F32, tag="iota85")
            nc.sync.dma_start(iota85[:], d["iota85"].ap())
            cembT = sbA.tile([CE, CV], F32, tag="cembT")
            nc.sync.dma_start(cembT[:], d["cembT"].ap())
            convT = sbA.tile([CE, 3 * CO], F32, tag="convT")
            nc.sync.dma_start(convT[:], d["convT"].ap())
            convb = sbA.tile([128, 1], F32, tag="convb")
            nc.sync.dma_start(convb[64:64 + CO, :], d["convb"].ap())

            ctab_ps = psA.tile([CV, 3 * CO], F32, space="PSUM", tag="ctab")
            nc.tensor.matmul(ctab_ps[:], cembT[:], convT[:], start=True, stop=True)
            ctab = sbA.tile([CV, 3 * CO], BF16, tag="ctabsb")
            nc.vector.tensor_copy(ctab[:], ctab_ps[:])

            onehot = sbA.tile([CV, NCH], BF16, tag="onehot")
            CHCH = 3840
            for c0 in range(0, NCH, CHCH):
                chbc = sbA.tile([CV, CHCH], BF16, tag="chb", bufs=2, name=f"chb{c0}")
                nc.sync.dma_start(chbc[:], _dap(d["charsb"], c0, [[0, CV], [1, CHCH]]))
                nc.vector.tensor_scalar(out=onehot[:, c0:c0 + CHCH], in0=chbc[:],
                                        scalar1=iota85[:], scalar2=None,
                                        op0=ALU.is_equal)

            for c in range(3):
                nc.vector.memset(embT[c][:], 0.0)

            widx = sbA.tile([128, 5], I32, tag="widx")
            nc.sync.dma_start(widx[:], _dap(d["widx"], 0, [[1, 128], [128, 5]]))
            for g in range(5):
                wrows = sbA.tile([128, 304], F32, tag="wrows", bufs=2, name=f"wrows{g}")
                nc.gpsimd.indirect_dma_start(
                    out=wrows[:, :300], out_offset=None, in_=d["word_emb"].ap(),
                    in_offset=bass.IndirectOffsetOnAxis(ap=widx[:, g:g + 1], axis=0))
                for c in range(3):
                    dim0 = c * 128
                    ncols = min(128, 300 - dim0)
                    tp = psA.tile([128, 128], F32, space="PSUM", tag="tp")
                    nc.tensor.transpose(tp[:ncols, :], wrows[:, dim0:dim0 + ncols],
                                        ident[:])
                    nc.vector.tensor_copy(embT[c][:ncols, g * 128:(g + 1) * 128],
                                          tp[:ncols, :128])
            nc.vector.memset(embT[2][96:97, :], 1.0)

            # charCNN: co[d,(t,j)] = sum_k Tk-matmul over windowed onehot
            TCW = 23
            oh3 = onehot[:].rearrange("c (t l) -> c t l", l=LP)
            nsl = (TEXT + TCW - 1) // TCW
            for sl in range(nsl):
                t0 = sl * TCW
                tn = min(TCW, TEXT - t0)
                co_ps = psA.tile([128, TCW * (L + 2)], F32, space="PSUM", tag="co")
                co3 = co_ps[64:64 + CO, :tn * (L + 2)].rearrange("d (t j) -> d t j", j=L + 2)
                for k3 in range(3):
                    nc.tensor.matmul(co3, ctab[:, k3 * CO:(k3 + 1) * CO],
                                     oh3[:, t0:t0 + tn, k3:k3 + L + 2],
                                     start=(k3 == 0), stop=(k3 == 2))
                cf = sbA.tile([128, TCW], F32, tag="cf", bufs=2, name=f"cf{sl}")
                nc.vector.tensor_reduce(cf[64:64 + CO, :tn], co3, axis=AX.X, op=ALU.max)
                nc.vector.tensor_scalar(out=embT[2][64:64 + CO, t0:t0 + tn],
                                        in0=cf[64:64 + CO, :tn],
                                        scalar1=convb[64:64 + CO, :],
                                        scalar2=None, op0=ALU.add)

            for c in range(3):
                nc.vector.tensor_copy(embTr[c][:], embT[c][:])

        # ---------- P5: input matmuls ----------
        with tc.tile_pool(name="psB", bufs=2, space="PSUM") as psB:
            for dd in range(2):
                for mc in range(8):
                    pc_ps = psB.tile([128, TEXT], F32, space="PSUM", tag="pcps")
                    for n0, nn in ((0, 512), (512, 128)):
                        for kc in range(3):
                            w0 = ((dd * 3 + kc) * 8 + mc) * 128
                            nc.tensor.matmul(pc_ps[:, n0:n0 + nn],
                                             w_ihT[:, w0:w0 + 128],
                                             embTr[kc][:, n0:n0 + nn],
                                             start=(kc == 0), stop=(kc == 2))
                    nc.scalar.activation(precompT[dd][:, mc * TEXT:(mc + 1) * TEXT],
                                         pc_ps[:], AF.Copy)

        # ---------- P6: LSTM supersteps ----------
        def pr_ap(dd, u):
            t3 = precompT[dd][:].rearrange("p (c r) -> p c r", c=8)
            if dd == 0:
                return t3[:, :, u:u + (S - 1) * B2 + 1:B2]
            base = PR - 1 - u
            return t3[:, :, base - (S - 1) * B2:base + 1:B2]

        def hv_ap(dd, u):
            t3 = hval[dd][:].rearrange("p (c r) -> p c r", c=2)
            if dd == 0:
                r0 = u - W
                return t3[:, :, r0:r0 + (S - 1) * B2 + 1:B2]
            base = BF - 1 - (u - W)
            return t3[:, :, base - (S - 1) * B2:base + 1:B2]

        with tc.tile_pool(name="psC", bufs=2, space="PSUM") as psC, \
             tc.tile_pool(name="sbC", bufs=1) as sbC:
            hscr = [sbC.tile([128, 2 * S], F32, tag=f"hscr{dd}", name=f"hscr{dd}") for dd in range(2)]
            hbf = [sbC.tile([128, 2 * S], BF16, tag=f"hbf{dd}", name=f"hbf{dd}") for dd in range(2)]
            cstA = [sbC.tile([128, 2 * S], F32, tag=f"cstA{dd}", name=f"cstA{dd}") for dd in range(2)]
            cstB = [sbC.tile([128, 2 * S], F32, tag=f"cstB{dd}", name=f"cstB{dd}") for dd in range(2)]
            gsb = [sbC.tile([128, 8 * S], F32, tag=f"gsb{dd}", name=f"gsb{dd}") for dd in range(2)]
            sgo = [sbC.tile([128, 6 * S], F32, tag=f"sgo{dd}", name=f"sgo{dd}") for dd in range(2)]
            tgg = [sbC.tile([128, 2 * S], F32, tag=f"tgg{dd}", name=f"tgg{dd}") for dd in range(2)]
            tg2 = [sbC.tile([128, 2 * S], F32, tag=f"tg2{dd}", name=f"tg2{dd}") for dd in range(2)]
            tm1 = [sbC.tile([128, 2 * S], F32, tag=f"tm1{dd}", name=f"tm1{dd}") for dd in range(2)]
            for dd in range(2):
                nc.vector.memset(hbf[dd][:], 0.0)
                nc.vector.memset(cstA[dd][:], 0.0)
                nc.vector.memset(hval[dd][:], 0.0)

            for u in range(NSUP):
                for dd in range(2):
                    gates = psC.tile([128, 8 * S], F32, space="PSUM", tag=f"g{dd}")
                    for mc in range(8):
                        for kc in range(2):
                            w0 = ((dd * 2 + kc) * 8 + mc) * 128
                            nc.tensor.matmul(gates[:, mc * S:(mc + 1) * S],
                                             w_hhT[:, w0:w0 + 128],
                                             hbf[dd][:, kc * S:(kc + 1) * S],
                                             start=(kc == 0), stop=(kc == 1))
                    g = gsb[dd]
                    nc.vector.tensor_tensor(
                        out=g[:].rearrange("p (c r) -> p c r", c=8),
                        in0=gates[:].rearrange("p (c r) -> p c r", c=8),
                        in1=pr_ap(dd, u), op=ALU.add)
                    nc.scalar.activation(sgo[dd][:], g[:, :6 * S], AF.Sigmoid)
                    nc.scalar.activation(tg2[dd][:], g[:, 6 * S:8 * S], AF.Tanh)
                    nc.vector.tensor_mul(tm1[dd][:], sgo[dd][:, :2 * S], tg2[dd][:])
                    nc.vector.tensor_mul(cstB[dd][:], sgo[dd][:, 2 * S:4 * S],
                                         cstA[dd][:])
                    nc.vector.tensor_add(cstA[dd][:], cstB[dd][:], tm1[dd][:])
                    nc.scalar.activation(tg2[dd][:], cstA[dd][:], AF.Tanh)
                    hdst = (hv_ap(dd, u) if u >= W
                            else hscr[dd][:].rearrange("p (c r) -> p c r", c=2))
                    nc.vector.tensor_tensor(
                        out=hdst,
                        in0=sgo[dd][:, 4 * S:6 * S].rearrange("p (c r) -> p c r", c=2),
                        in1=tg2[dd][:].rearrange("p (c r) -> p c r", c=2), op=ALU.mult)
                    nc.scalar.activation(
                        hbf[dd][:].rearrange("p (c r) -> p c r", c=2), hdst, AF.Copy)

        # ---------- P7: feats ----------
        with tc.tile_pool(name="psD", bufs=2, space="PSUM") as psD, \
             tc.tile_pool(name="sbD", bufs=2) as sbD:
            fsb = sb.tile([16, FPAD + 512], F32)
            nc.vector.memset(fsb[:], 0.0)
            f_ps = psD.tile([16, 448], F32, space="PSUM", tag="fps")
            first = True
            for dd in range(2):
                for hc in range(2):
                    wt = h2tT[:, (dd * 2 + hc) * 16:(dd * 2 + hc) * 16 + 16]
                    nc.tensor.matmul(f_ps[:, :BF], wt, hval[dd][:, hc * BF:(hc + 1) * BF],
                                     start=first, stop=(dd == 1 and hc == 1))
                    first = False
            nc.vector.tensor_copy(fsb[:, FPAD:FPAD + BF], f_ps[:, :BF])
            fz = sb.tile([128, K], F32)
            nc.vector.memset(fz[:], 0.0)
            nc.sync.dma_start(_dap(feats_d, 0, [[K, FPAD], [1, K]]), fz[:FPAD, :])
            nc.sync.dma_start(_dap(feats_d, (FPAD + BF) * K, [[K, FD - FPAD - BF], [1, K]]),
                              fz[:FD - FPAD - BF, :])
            for g in range(4):
                tp2 = psD.tile([128, 16], F32, space="PSUM", tag="ftp")
                nc.tensor.transpose(tp2[:, :16], fsb[:16, FPAD + g * 128:FPAD + (g + 1) * 128],
                                    ident[:16, :16])
                ftile = sbD.tile([128, K], F32, tag="ftile")
                nc.vector.tensor_copy(ftile[:], tp2[:, :K])
                nc.sync.dma_start(
                    _dap(feats_d, (FPAD + g * 128) * K, [[K, 128], [1, K]]), ftile[:])

        # ---------- P8: viterbi feats views + patch ----------
        NV = NSUPV
        with tc.tile_pool(name="sbE", bufs=1) as sbE, \
             tc.tile_pool(name="psE", bufs=2, space="PSUM") as psE:
            fva = sbE.tile([128, NV * K], F32, tag="fva")
            fvb = sbE.tile([128, NV * K], F32, tag="fvb")
            vscr = sbE.tile([128, NV * K], F32, tag="vscr")
            vmsk = sbE.tile([128, NV * K], F32, tag="vmsk")
            vpat = sbE.tile([128, NV * K], F32, tag="vpat")
            AOFF, ASTEP, AUST = FPAD * K, B2V * K, K
            BOFF, BSTEP, BUST = (FPAD + BROW) * K, B2V * K, K
            for vt, off, pstep, ustep in ((fva, AOFF, ASTEP, AUST),
                                          (fvb, BOFF, BSTEP, BUST)):
                nc.sync.dma_start(vscr[:].rearrange("p (u k) -> p u k", k=K),
                                  _dap(feats_d, off, [[pstep, 128], [ustep, NV], [1, K]]))
                nc.sync.dma_start(vmsk[:].rearrange("p (u k) -> p u k", k=K),
                                  _dap(d["fmask"], off, [[pstep, 128], [ustep, NV], [1, K]]))
                nc.sync.dma_start(vpat[:].rearrange("p (u k) -> p u k", k=K),
                                  _dap(d["fpatch"], off, [[pstep, 128], [ustep, NV], [1, K]]))
                nc.vector.tensor_mul(vmsk[:], vscr[:], vmsk[:])
                nc.vector.tensor_add(vt[:], vmsk[:], vpat[:])

            # ---------- P9: alpha/beta scans ----------
            absb = sbE.tile([128, 2 * 3 * K], F32, tag="absb")
            for which, fview, trr in ((0, fva, trrep), (1, fvb, trrepb)):
                fv = sbE.tile([128, K], F32, tag=f"fv{which}", name=f"fv{which}")
                fvR = sbE.tile([128, K], F32, tag=f"fvR{which}", name=f"fvR{which}")
                nc.vector.memset(fv[:], 0.0)
                tmpv = sbE.tile([128, K * K], F32, tag=f"tmpv{which}", name=f"tmpv{which}")
                f3 = fview[:].rearrange("p (u k) -> p u k", k=K)
                eng = nc.vector
                for u in range(NV):
                    uf = u if which == 0 else (NV - 1 - u)
                    eng.tensor_tensor(
                        out=tmpv[:].rearrange("p (i j) -> p i j", i=K),
                        in0=fv[:].rearrange("p (o j) -> p o j", o=1).to_broadcast([128, K, K]),
                        in1=trr[:].rearrange("p (i j) -> p i j", i=K), op=ALU.add)
                    nc.vector.tensor_reduce(fvR[:], tmpv[:].rearrange(
                        "p (i j) -> p i j", i=K), axis=AX.X, op=ALU.max)
                    eng.tensor_tensor(out=fv[:], in0=fvR[:], in1=f3[:, uf, :],
                                      op=ALU.add)
                    if WV <= u < WV + 3:
                        slot = which * 3 + ((u - WV) if which == 0 else (WV + 2 - u))
                        nc.vector.tensor_copy(absb[:, slot * K:(slot + 1) * K], fv[:])
            zt = sbE.tile([128, K], F32, tag="zt")
            nc.vector.memset(zt[:], 0.0)
            for which, zhi in ((0, FPAD + WV), (1, FPAD + BSROW)):
                base = which * FD * K
                for z0 in range(0, zhi, 128):
                    zn = min(128, zhi - z0)
                    nc.sync.dma_start(_dap(ab_d, base + z0 * K, [[K, zn], [1, K]]),
                                      zt[:zn, :])
                nc.sync.dma_start(
                    _dap(ab_d, base + (FPAD + BF) * K, [[K, FD - FPAD - BF], [1, K]]),
                    zt[:FD - FPAD - BF, :])
            # alpha rows: n*3 + 56 + s ; beta rows: 390 - 3n - s
            nc.sync.dma_start(
                _dap(ab_d, (FPAD + WV) * K, [[3 * K, 128], [K, 3], [1, K]]),
                absb[:, 0:3 * K].rearrange("p (s k) -> p s k", k=K))
            nc.sync.dma_start(
                _dap(ab_d, FD * K + (FPAD + BSROW) * K, [[3 * K, 128], [K, 3], [1, K]]),
                absb[:, 3 * K:6 * K].rearrange("p (s k) -> p s k", k=K))

            # ---------- P10: path + score ----------
            def aligned(src, base, shift):
                return _dap(src, base + (FPAD + shift) * K, [[K, 128], [128 * K, 3], [1, K]])

            al_al = sbE.tile([128, 3 * K], F32, tag="alal")
            be_al = sbE.tile([128, 3 * K], F32, tag="beal")
            al_p = sbE.tile([128, 3 * K], F32, tag="alp")
            be_p = sbE.tile([128, 3 * K], F32, tag="bep")
            ft_al = sbE.tile([128, 3 * K], F32, tag="ftal")
            ft_p = sbE.tile([128, 3 * K], F32, tag="ftp2")
            for dst, src, base, sh in ((al_al, ab_d, 0, 0), (be_al, ab_d, FD * K, 0),
                                       (al_p, ab_d, 0, -1), (be_p, ab_d, FD * K, -1),
                                       (ft_al, feats_d, 0, 0), (ft_p, feats_d, 0, -1)):
                nc.sync.dma_start(dst[:].rearrange("p (g k) -> p g k", k=K),
                                  aligned(src, base, sh))
            tot = sbE.tile([128, 3 * K], F32, tag="tot")
            totp = sbE.tile([128, 3 * K], F32, tag="totp")
            gsc = sbE.tile([128, 3 * K], F32, tag="gsc")
            # tot = alpha + (gamma - feats)   (beta scan state is gamma = beta + feats)
            nc.vector.tensor_tensor(out=gsc[:], in0=be_al[:], in1=ft_al[:], op=ALU.subtract)
            nc.vector.tensor_add(tot[:], al_al[:], gsc[:])
            nc.vector.tensor_tensor(out=gsc[:], in0=be_p[:], in1=ft_p[:], op=ALU.subtract)
            nc.vector.tensor_add(totp[:], al_p[:], gsc[:])
            mx = sbE.tile([128, 3], F32, tag="mx")
            mxp = sbE.tile([128, 3], F32, tag="mxp")
            msk = sbE.tile([128, 3 * K], F32, tag="msk")
            mskp = sbE.tile([128, 3 * K], F32, tag="mskp")
            for mm, tt2, mk in ((mx, tot, msk), (mxp, totp, mskp)):
                nc.vector.tensor_reduce(mm[:], tt2[:].rearrange("p (g k) -> p g k", k=K),
                                        axis=AX.X, op=ALU.max)
                nc.vector.tensor_tensor(
                    out=mk[:].rearrange("p (g k) -> p g k", k=K),
                    in0=tt2[:].rearrange("p (g k) -> p g k", k=K),
                    in1=mm[:].rearrange("p (g o) -> p g o", o=1).to_broadcast([128, 3, K]),
                    op=ALU.is_ge)
            iotaK = sbE.tile([128, K], F32, tag="iotaK")
            nc.gpsimd.iota(iotaK[:], pattern=[[1, K]], base=0, channel_multiplier=0,
                           allow_small_or_imprecise_dtypes=True)
            wrk = sbE.tile([128, 3 * K], F32, tag="wrk")
            nc.vector.tensor_scalar(out=wrk[:], in0=iotaK[:].rearrange(
                "p (o k) -> p o k", o=1).to_broadcast([128, 3, K]), scalar1=-1.0,
                scalar2=float(K - 1), op0=ALU.mult, op1=ALU.add)
            wrk2 = sbE.tile([128, 3 * K], F32, tag="wrk2")
            nc.vector.tensor_mul(wrk2[:], wrk[:], msk[:])
            pathf = sbE.tile([128, 3], F32, tag="pathf")
            pathf2 = sbE.tile([128, 3], F32, tag="pathf2")
            nc.vector.tensor_reduce(pathf2[:], wrk2[:].rearrange("p (g k) -> p g k", k=K),
                                    axis=AX.X, op=ALU.max)
            nc.vector.tensor_scalar(out=pathf[:], in0=pathf2[:], scalar1=-1.0,
                                    scalar2=float(K - 1), op0=ALU.mult, op1=ALU.add)
            pathi = sbE.tile([128, 3], I32, tag="pathi")
            nc.vector.tensor_copy(pathi[:], pathf[:])
            # rows r in [57, 313) -> path_o[0:256]
            n0 = 128 - RV0
            nc.sync.dma_start(_dap(path_o, 0, [[1, n0]]), pathi[RV0:128, 0:1])
            nc.sync.dma_start(_dap(path_o, n0, [[1, 128]]), pathi[:, 1:2])
            nc.sync.dma_start(_dap(path_o, n0 + 128, [[1, RV0]]), pathi[0:RV0, 2:3])

            # score partials
            sc0 = sbE.tile([128, 3 * K], F32, tag="sc0")
            sc1 = sbE.tile([128, 3 * K], F32, tag="sc1")
            nc.vector.tensor_mul(sc0[:], ft_al[:], msk[:])
            nc.vector.tensor_tensor(
                out=sc1[:].rearrange("p (g k) -> p g k", k=K),
                in0=sc0[:].rearrange("p (g k) -> p g k", k=K),
                in1=vmask1[:].rearrange("p (g o) -> p g o", o=1).to_broadcast([128, 3, K]),
                op=ALU.mult)
            w12 = sbE.tile([128, 3 * K * K], F32, tag="w12")
            nc.vector.tensor_tensor(
                out=w12[:].rearrange("p (g i j) -> p g i j", i=K, j=K),
                in0=trrep[:].rearrange("p (o i j) -> p o i j", o=1, i=K).to_broadcast([128, 3, K, K]),
                in1=mskp[:].rearrange("p (g o j) -> p g o j", o=1, j=K).to_broadcast([128, 3, K, K]), op=ALU.mult)
            wred0 = sbE.tile([128, 3 * K], F32, tag="wred0")
            wred1 = sbE.tile([128, 3 * K], F32, tag="wred1")
            wred = sbE.tile([128, 3 * K], F32, tag="wred")
            nc.vector.tensor_reduce(wred0[:], w12[:].rearrange(
                "p (g i j) -> p g i j", i=K, j=K), axis=AX.X, op=ALU.add)
            nc.vector.tensor_mul(wred1[:], wred0[:], msk[:])
            nc.vector.tensor_tensor(
                out=wred[:].rearrange("p (g k) -> p g k", k=K),
                in0=wred1[:].rearrange("p (g k) -> p g k", k=K),
                in1=vmask2[:].rearrange("p (g o) -> p g o", o=1).to_broadcast([128, 3, K]),
                op=ALU.mult)
            s3 = sbE.tile([128, K], F32, tag="s3")
            nc.vector.tensor_mul(s3[:], trstop[:], msk[:, 2 * K:3 * K])
            acc = sbE.tile([128, 3 * K], F32, tag="acc")
            nc.vector.tensor_add(acc[:], sc1[:], wred[:])
            stot = sbE.tile([128, 1], F32, tag="stot")
            nc.vector.tensor_reduce(stot[:], acc[:], axis=AX.X, op=ALU.add)
            s3r = sbE.tile([128, 1], F32, tag="s3r")
            s3m = sbE.tile([128, 1], F32, tag="s3m")
            stot2 = sbE.tile([128, 1], F32, tag="stot2")
            nc.vector.tensor_reduce(s3r[:], s3[:], axis=AX.X, op=ALU.add)
            nc.vector.tensor_mul(s3m[:], s3r[:], vmask3[:])
            nc.vector.tensor_add(stot2[:], stot[:], s3m[:])
            nc.sync.dma_start(score_o.ap(), stot2[:])

            if debug_outputs:
                nc.sync.dma_start(dbg["feats_o"].ap(), feats_d.ap())
                for c in range(3):
                    nc.sync.dma_start(dbg["emb_o"].ap()[c], embT[c][:])
                for dd2 in range(2):
                    nc.sync.dma_start(dbg["h_o"].ap()[dd2], hval[dd2][:])
                nc.sync.dma_start(dbg["ab_o"].ap(), ab_d.ap())

    nc.compile()
    return nc


# ---------------- host-side prep ----------------
def make_in_maps(inputs):
    inputs = {k: np.asarray(v) for k, v in inputs.items()}
    sentence = inputs["sentence"].astype(np.int64)
    chars = inputs["chars"].astype(np.int64)
    word_emb = np.concatenate(
        [inputs["word_emb"].astype(np.float32), np.zeros((1, E), np.float32)], 0)
    import ml_dtypes

    # shared param layouts
    CHUNKMAP = [0, 1, 2, 3, 6, 7, 4, 5]   # device gate chunks: i,f,o,g order

    def lhsT_tiles(w, kdim, kchunks, mchunks):
        # w: [out(gate), in] -> tiles [(kc, mc)] each [128(K), 128(M)]
        out = np.zeros((128, kchunks * mchunks * 128), np.float32)
        for kc in range(kchunks):
            for mc in range(mchunks):
                wb = CHUNKMAP[mc] if mchunks == 8 else mc
                blk = w[wb * 128:(wb + 1) * 128, kc * 128:(kc + 1) * 128]
                out[:, (kc * mchunks + mc) * 128:(kc * mchunks + mc) * 128 + 128] = blk.T
        return out

    w_ih_ext = {}
    for dd, (wn, bn) in enumerate((("w_ih_f", "b_f"), ("w_ih_b", "b_b"))):
        wext = np.zeros((4 * H, 384), np.float32)
        wext[:, :300] = inputs[wn][:, :300]
        wext[:, 320:345] = inputs[wn][:, 300:325]
        wext[:, 352] = inputs[bn]
        w_ih_ext[dd] = wext
    w_ihT = np.concatenate([lhsT_tiles(w_ih_ext[dd], 384, 3, 8) for dd in range(2)], 1)
    w_hhT = np.concatenate(
        [lhsT_tiles(inputs[wn], 256, 2, 8) for wn in ("w_hh_f", "w_hh_b")], 1
    ).astype(ml_dtypes.bfloat16)
    h2tT = np.zeros((128, 2 * 2 * 16), np.float32)
    for dd in range(2):
        for hc in range(2):
            blk = inputs["h2t_w"][:, dd * 256 + hc * 128: dd * 256 + (hc + 1) * 128]
            h2tT[:, (dd * 2 + hc) * 16:(dd * 2 + hc) * 16 + 12] = blk.T
    cembT = inputs["char_emb"].astype(np.float32).T.copy()           # [25, 85]
    convT = inputs["conv_w"][:, 0, :, :].transpose(2, 1, 0).reshape(CE, 3 * CO).copy()
    convb = inputs["conv_b"].astype(np.float32).reshape(CO, 1)
    trans = inputs["transitions"].astype(np.float32)
    trrep = np.tile(trans.reshape(1, K * K), (128, 1))
    ident = np.eye(128, dtype=np.float32)
    iota85 = np.arange(CV).reshape(CV, 1).astype(np.float32)

    in_maps = []
    for k in range(NC):
        m = {"word_emb": word_emb, "w_ihT": w_ihT, "w_hhT": w_hhT, "h2tT": h2tT,
             "cembT": cembT, "convT": convT, "convb": convb, "trrep": trrep,
             "ident": ident, "iota85": iota85}
        # gather indices + char grid
        widx = np.full((TEXT, 1), V, np.int32)
        charsb = np.full((TEXT, LP), PADCHAR, np.float32)
        for pr in range(TEXT):
            t = pr2t(k, pr)
            if 0 <= t < T:
                widx[pr, 0] = sentence[t]
                charsb[pr, 2:2 + L] = chars[t]
        m["widx"] = widx
        m["charsb"] = charsb.reshape(-1).astype(ml_dtypes.bfloat16)
        # beta transitions: transposed; core 7 masked for terminal STOP selection
        trb = trans.T.copy()
        if k == NC - 1:
            trb[START, STOP] += NEG
            trb[STOP, STOP] += NEG
        m["trrepb"] = np.tile(trb.reshape(1, K * K), (128, 1))
        # feats mask/patch
        fmask = np.ones((FD, K), np.float32)
        fpatch = np.zeros((FD, K), np.float32)
        if k == 0:
            fmask[FPAD + RV0 - 1] = 0.0
            fpatch[FPAD + RV0 - 1] = NEG
            fpatch[FPAD + RV0 - 1, START] = 0.0
        if k == NC - 1:
            r_T = (T - row2t(k, 0))  # feats row for t == T
            fmask[FPAD + r_T] = 0.0
            fpatch[FPAD + r_T] = NEG
            fpatch[FPAD + r_T, STOP] = 0.0
        m["fmask"] = fmask
        m["fpatch"] = fpatch
        m["trstop"] = (np.tile(trans[STOP].reshape(1, K), (128, 1))
                       if k == NC - 1 else np.zeros((128, K), np.float32))
        vm1 = np.zeros((128, 3), np.float32)
        vm2 = np.zeros((128, 3), np.float32)
        for g in range(3):
            for p in range(128):
                r = g * 128 + p
                if RV0 <= r < RV0 + B:
                    vm1[p, g] = 1.0
                    vm2[p, g] = 1.0
        m["vmask1"] = vm1
        m["vmask2"] = vm2
        vm3 = np.zeros((128, 1), np.float32)
        if k == NC - 1:
            vm3[RV0 - 1, 0] = 1.0
        m["vmask3"] = vm3
        in_maps.append(m)
    return in_maps


_NC_CACHE = {}


def kernel(**inputs):
    from concourse import bass_utils
    key = "main"
    if key not in _NC_CACHE:
        _NC_CACHE[key] = build_nc(debug_outputs=False)
    nc = _NC_CACHE[key]
    in_maps = make_in_maps(inputs)
    res = bass_utils.run_bass_kernel_spmd(nc, in_maps, core_ids=list(range(NC)))
    path = np.concatenate([res.results[k]["path_o"] for k in range(NC)]).astype(np.int32)
    score = np.float32(sum(np.float32(res.results[k]["score_o"].sum()) for k in range(NC)))
    return score, path
